# revision 1
# baseline (speedup 1.0000x reference)
"""Trainium2 Bass kernel for nn_AttentionBlock (B=8, L=2048, C=512, GroupNorm(8) +
single-head attention + residual), data-parallel over batch across 8 NeuronCores.

Self-contained: hardcodes shapes/sharding. kernel(**inputs) -> np.ndarray [B,L,C].

Dataflow (per core, one batch element, everything channel-major / "transposed"):
  x^T [C,L] (f32) --bn_stats/group-reduce--> h^T = a_c * x^T + b_c  (f32 + bf16 copy)
  Q^T = wq^T h^T + bq ;  K^T = (wk*scale)^T h^T + bk*scale  (scale folded on host)
  V   = h^T-chunks^T @ wv + bv           (natural [L,C] layout)
  per 512-wide lq tile:
     for each 128-key block: S^T = K^T-chunk^T @ Q^T (PSUM f32); P = exp(S^T) (bf16)
     O^T  += V-chunk^T @ P  (PSUM f32 accum over key blocks), denom += 1^T @ P
     out^T = h^T + (wp^T O^T) * (1/denom) + bp      (f32 combine)
Matmul operands are bf16 (1 cyc/row on PE); accumulation always fp32 in PSUM.
Host side transposes x per batch, casts weights to bf16, transposes output back.
"""

import numpy as np

B, L, C = 8, 2048, 512
GROUPS = 8
EPS = 1e-3
P = 128
CS = C // P            # 4 channel subtiles of 128
LQ = 512               # lq tile width (matmul free dim)
NLT = L // LQ          # 4 lq tiles
NLB = L // P           # 16 key/l blocks
CPG = C // GROUPS      # 64 channels per group
N_CORES = 8

_CACHE = {}


def _build_nc():
    from contextlib import ExitStack

    import concourse.bass as bass
    import concourse.mybir as mybir
    import concourse.tile as tile
    from concourse import bacc
    from concourse.bass import ts

    f32 = mybir.dt.float32
    bf16 = mybir.dt.bfloat16
    fp8 = mybir.dt.float8e4
    DR = mybir.MatmulPerfMode.DoubleRow
    AF = mybir.ActivationFunctionType
    ALU = mybir.AluOpType

    nc = bacc.Bacc(trn_type="TRN2")

    xr_d = nc.dram_tensor("xr", [NLT, P, CS, LQ], f32, kind="ExternalInput")
    xb_d = nc.dram_tensor("xb", [C, L], bf16, kind="ExternalInput")
    w_d = {
        n: nc.dram_tensor(n, [P, CS, C], fp8, kind="ExternalInput")
        for n in ("wq", "wk", "wv", "wp")
    }
    # packed per-channel vectors: [gamma, beta, bq, bk, bp] x CS columns
    vp_d = nc.dram_tensor("vp", [P, 5 * CS], f32, kind="ExternalInput")
    bvb_d = nc.dram_tensor("bv_bcast", [P, C], f32, kind="ExternalInput")
    g0_d = nc.dram_tensor("g0", [P, 2], f32, kind="ExternalInput")
    sel_d = nc.dram_tensor("sel", [2, P], f32, kind="ExternalInput")
    out_d = nc.dram_tensor("out_t", [C, L], f32, kind="ExternalOutput")

    xb_dv = xb_d[:].rearrange("(s p) l -> p s l", p=P)
    out_dv = out_d[:].rearrange("(s p) l -> p s l", p=P)

    with tile.TileContext(nc) as tc, ExitStack() as ctx:
        consts = ctx.enter_context(tc.tile_pool(name="consts", bufs=1))
        data = ctx.enter_context(tc.tile_pool(name="data", bufs=1))
        small = ctx.enter_context(tc.tile_pool(name="small", bufs=1))
        ptp = ctx.enter_context(tc.tile_pool(name="ptp", bufs=3))
        oup = ctx.enter_context(tc.tile_pool(name="oup", bufs=4))
        finp = ctx.enter_context(tc.tile_pool(name="finp", bufs=2))
        psA = ctx.enter_context(tc.tile_pool(name="psA", bufs=4, space="PSUM"))
        psS = ctx.enter_context(tc.tile_pool(name="psS", bufs=3, space="PSUM"))
        psD = ctx.enter_context(tc.tile_pool(name="psD", bufs=1, space="PSUM"))

        # ---- SBUF residents ----
        xt = data.tile([P, CS, L], f32)       # x^T, then h^T (f32, residual)
        xb = data.tile([P, CS, L], bf16)      # x^T bf16, stats fast path
        hb = data.tile([P, CS, L], fp8)       # h^T fp8 (matmul operand)
        qt = data.tile([P, CS, L], fp8)       # Q^T (fp8: attention matmuls run
        kt = data.tile([P, CS, L], fp8)       # DoubleRow, 2x PE throughput)
        vt = data.tile([P, NLB, C], fp8)      # V natural, [l%P, l//P, c]
        wsb = {n: consts.tile([P, CS, C], fp8, name=f"w_{n}") for n in w_d}
        vp = consts.tile([P, 5 * CS], f32)
        GAM, BET, BQ, BK, BP = (vp[:, i * CS:(i + 1) * CS] for i in range(5))
        bvb = consts.tile([P, C], f32)
        g0 = consts.tile([P, 2], f32)
        sel = consts.tile([2, P], f32)
        ones_col = consts.tile([P, 2, 16], fp8)   # [:, :, 0:1] = DoubleRow ones
        ones_row = consts.tile([1, P], bf16)
        eps2 = consts.tile([2, 1], f32)

        # ---- loads + constants ----
        # DMA order IS the critical path: tiny constants first (they gate the
        # stats reduce), then bf16 x (gates bn_stats), then weights (gate the
        # first projection matmuls), then f32 x (residual only — needed late).
        nc.gpsimd.dma_start(out=g0[:], in_=g0_d[:])
        nc.gpsimd.dma_start(out=sel[:], in_=sel_d[:])
        nc.gpsimd.dma_start(out=vp[:], in_=vp_d[:])
        for s in range(CS):
            nc.sync.dma_start(out=xb[:, s, :], in_=xb_dv[:, s, :])
        for n in ("wq", "wk", "wv", "wp"):
            nc.sync.dma_start(out=wsb[n][:], in_=w_d[n][:])
        nc.sync.dma_start(out=bvb[:], in_=bvb_d[:])
        # f32 x only feeds the residual add in the lt-th finale — stream it
        # per lq tile so it never competes with the latency-critical loads.
        for lt in range(NLT):
            nc.sync.dma_start(out=xt[:, :, ts(lt, LQ)], in_=xr_d[lt])
        nc.vector.memset(ones_col[:], 1.0)
        nc.vector.memset(ones_row[:], 1.0)
        nc.vector.memset(eps2[:], EPS)

        # ---- GroupNorm stats ----
        # per-channel (partition) mean/var over L via bn_stats, then group
        # aggregation across partitions with a tiny fp32 matmul.
        st = small.tile([P, CS, 2], f32)      # (mean_c, E[x^2]_c) per subtile
        # subtiles 0..2 on DVE (bn_stats); subtile 3 on the otherwise-idle ACT
        # via activation accum_out (sum and sum-of-squares along L).
        for s in range(CS - 1):
            st6 = small.tile([P, 4, 6], f32, tag="st6", bufs=2)
            for j in range(4):
                nc.vector.bn_stats(out=st6[:, j, :], in_=xb[:, s, ts(j, 512)])
            mv = small.tile([P, 2], f32, tag="mv", bufs=2)
            nc.vector.bn_aggr(out=mv[:], in_=st6[:])
            nc.vector.tensor_copy(out=st[:, s, 0:1], in_=mv[:, 0:1])
            nc.vector.tensor_tensor(out=st[:, s, 1:2], in0=mv[:, 0:1], in1=mv[:, 0:1], op=ALU.mult)
            nc.vector.tensor_tensor(out=st[:, s, 1:2], in0=st[:, s, 1:2], in1=mv[:, 1:2], op=ALU.add)
        s3 = CS - 1
        ssum = small.tile([P, 2], f32)        # (sum, sumsq) of subtile 3
        gscr = small.tile([P, L], bf16)
        nc.scalar.activation(out=gscr[:], in_=xb[:, s3, :], func=AF.Identity,
                             accum_out=ssum[:, 0:1])
        nc.scalar.activation(out=gscr[:], in_=xb[:, s3, :], func=AF.Square,
                             accum_out=ssum[:, 1:2])
        nc.vector.tensor_scalar(out=st[:, s3, :], in0=ssum[:], scalar1=1.0 / L, scalar2=None,
                                op0=ALU.mult)

        psg = psD.tile([2, 2 * CS], f32, tag="d")   # [group-half, (s, stat)]
        nc.tensor.matmul(psg[:], lhsT=g0[:], rhs=st[:].rearrange("p a b -> p (a b)"),
                         start=True, stop=True)
        pst = small.tile([2, 2 * CS], f32)
        nc.vector.tensor_copy(out=pst[:], in_=psg[:])
        pstv = pst[:].rearrange("p (s k) -> p s k", k=2)
        msq = small.tile([2, CS], f32)
        nc.vector.tensor_tensor(out=msq[:], in0=pstv[:, :, 0], in1=pstv[:, :, 0], op=ALU.mult)
        grp = small.tile([2, 2 * CS], f32)     # [:, :CS]=rstd_g, [:, CS:]=mean_g
        nc.vector.tensor_tensor(out=grp[:, 0:CS], in0=pstv[:, :, 1], in1=msq[:], op=ALU.subtract)
        nc.scalar.activation(out=grp[:, 0:CS], in_=grp[:, 0:CS], func=AF.Sqrt,
                             bias=eps2[:], scale=1.0)
        # plain DVE reciprocal has a ~2.7us floor; the ~18-bit approx is exact
        # enough for rstd (inputs are sqrt(var+eps), always normal fp32)
        nc.vector.reciprocal_approx_fast(out=grp[:, 0:CS], in_=grp[:, 0:CS])
        nc.vector.tensor_copy(out=grp[:, CS:], in_=pstv[:, :, 0])
        # dummy Exp: pulls the Exp table-set load (~2.7us) off the first real
        # exp's critical path; Identity (used by the Q/K copies) is a filler
        # function present in every set.
        nc.scalar.activation(out=msq[:, 0:1], in_=eps2[:], func=AF.Exp)

        psbc = psD.tile([P, 2 * CS], f32, tag="d")  # broadcast groups -> channels
        nc.tensor.matmul(psbc[:], lhsT=sel[:], rhs=grp[:], start=True, stop=True)
        ab = small.tile([P, 2 * CS], f32)      # [:, :CS]=a_c, [:, CS:]=b_c
        nc.vector.tensor_tensor(out=ab[:, 0:CS], in0=GAM, in1=psbc[:, 0:CS], op=ALU.mult)
        nc.vector.tensor_tensor(out=ab[:, CS:], in0=psbc[:, CS:], in1=ab[:, 0:CS], op=ALU.mult)
        nc.vector.tensor_tensor(out=ab[:, CS:], in0=BET, in1=ab[:, CS:], op=ALU.subtract)
        # residual pass scalars with the output-projection bias folded in:
        # out = (a*x + b + bp) + Z/denom
        ab2 = small.tile([P, CS], f32)
        nc.vector.tensor_tensor(out=ab2[:], in0=ab[:, CS:], in1=BP, op=ALU.add)

        # ---- normalize: h^T = a*x^T + b ----
        # bf16 copy first (it gates all matmuls), split across DVE and ACT so
        # all four subtiles are ready ~2x sooner; the f32 in-place pass only
        # feeds the residual add much later, so it runs off the critical path.
        for s in range(CS):
            if s < 2:
                nc.vector.tensor_scalar(out=hb[:, s, :], in0=xb[:, s, :],
                                        scalar1=ab[:, s:s + 1], scalar2=ab[:, CS + s:CS + s + 1],
                                        op0=ALU.mult, op1=ALU.add)
            else:
                nc.scalar.activation(out=hb[:, s, :], in_=xb[:, s, :], func=AF.Identity,
                                     bias=ab[:, CS + s:CS + s + 1], scale=ab[:, s:s + 1])
        def residual_pass(lt):
            # h^T + bp for the lt-th finale, in place over the streamed f32 x
            for s in range(CS):
                nc.vector.tensor_scalar(out=xt[:, s, ts(lt, LQ)], in0=xt[:, s, ts(lt, LQ)],
                                        scalar1=ab[:, s:s + 1], scalar2=ab2[:, s:s + 1],
                                        op0=ALU.mult, op1=ALU.add)

        # ---- projections ----
        def project_t(w, bias, dst, on_act):
            # dst[:, co_s, l] = sum_ci w[ci, co]^T h^T + bias[co]; weights come
            # in x8 (fp8 range), the copy rescales by 1/8. Copies split across
            # ACT and DVE so neither gates the projection phase.
            for co_s in range(CS):
                for lt in range(NLT):
                    ps = psS.tile([P, LQ], f32, tag="s", name="ps_prj")
                    for cp in range(2):
                        nc.tensor.matmul(ps[:], lhsT=w[:, 2 * cp:2 * cp + 2, ts(co_s, P)],
                                         rhs=hb[:, 2 * cp:2 * cp + 2, ts(lt, LQ)],
                                         start=(cp == 0), stop=(cp == 1), perf_mode=DR)
                    if on_act:
                        nc.scalar.activation(out=dst[:, co_s, ts(lt, LQ)], in_=ps[:],
                                             func=AF.Identity, bias=bias[:, co_s:co_s + 1],
                                             scale=1.0 / 8)
                    else:
                        nc.vector.tensor_scalar(out=dst[:, co_s, ts(lt, LQ)], in0=ps[:],
                                                scalar1=1.0 / 8, scalar2=bias[:, co_s:co_s + 1],
                                                op0=ALU.mult, op1=ALU.add)

        project_t(wsb["wq"], BQ, qt, on_act=False)
        project_t(wsb["wk"], BK, kt, on_act=True)

        for lb in range(NLB):
            ps = psS.tile([P, C], f32, tag="s", name="ps_v")
            for cp in range(2):
                nc.tensor.matmul(ps[:], lhsT=hb[:, 2 * cp:2 * cp + 2, ts(lb, P)],
                                 rhs=wsb["wv"][:, 2 * cp:2 * cp + 2, :],
                                 start=(cp == 0), stop=(cp == 1), perf_mode=DR)
            # V stays scaled x4 (wv, bv x4 on host); the 4x8=32 factor from
            # V and wp is divided out of the softmax denominators below.
            nc.vector.tensor_add(out=vt[:, lb, :], in0=ps[:], in1=bvb[:])

        # ---- attention + output projection, per lq tile ----
        for lt in range(NLT):
            po = [psA.tile([P, LQ], f32, tag="po", name=f"po{i}") for i in range(CS)]
            pd = psD.tile([1, LQ], f32, tag="d", name="pd")
            # Software-pipelined: each PV/denominator group is emitted one
            # key-pair behind its S^T/exp group, so the in-order PE always has
            # S-work queued ahead of a PV matmul that may wait on a PSUM bank.
            def pv_group(kp, pt2):
                for c_ in range(CS):
                    nc.tensor.matmul(po[c_][:], lhsT=vt[:, 2 * kp:2 * kp + 2, ts(c_, P)],
                                     rhs=pt2[:], start=(kp == 0), stop=(kp == NLB // 2 - 1),
                                     perf_mode=DR)
                nc.tensor.matmul(pd[:], lhsT=ones_col[:, :, 0:1], rhs=pt2[:],
                                 start=(kp == 0), stop=(kp == NLB // 2 - 1), perf_mode=DR)

            prev = None
            for kp in range(NLB // 2):
                # S^T for a pair of key blocks: 2 DoubleRow matmuls each
                # (contraction 256 = two channel subtiles per matmul)
                pt2 = ptp.tile([P, 2, LQ], fp8, tag="pt")
                for i in range(2):
                    kb = 2 * kp + i
                    ps = psS.tile([P, LQ], f32, tag="s", name="ps_s")
                    for cp in range(2):
                        nc.tensor.matmul(ps[:], lhsT=kt[:, 2 * cp:2 * cp + 2, ts(kb, P)],
                                         rhs=qt[:, 2 * cp:2 * cp + 2, ts(lt, LQ)],
                                         start=(cp == 0), stop=(cp == 1), perf_mode=DR)
                    nc.scalar.activation(out=pt2[:, i, :], in_=ps[:], func=AF.Exp)
                if prev is not None:
                    pv_group(*prev)
                prev = (kp, pt2)
            pv_group(*prev)

            # Finale. Order matters: pdc frees the "d" bank and the ou copies
            # free the "po" banks that the next lq tile's denominator/PV
            # matmuls need — emit them first so DVE runs them first.
            # Broadcast raw denominators across partitions via PE, then take
            # the reciprocal on all 128 lanes (a [1,512] single-lane
            # reciprocal is ~2.7us and stalls the PE).
            # Fold 1/denom into the O^T cast: ou = O * (64/denom) stays in
            # fp8's normal range, the output-projection matmuls then produce
            # the final attention term directly (scaled 512x), and the
            # post-wp DVE chain is a single fused op per chunk.
            pdc = small.tile([1, LQ], bf16, tag="pdc", bufs=2)
            with nc.allow_low_precision(reason="denom rounded to bf16 as matmul operand"):
                nc.vector.tensor_scalar(out=pdc[:], in0=pd[:], scalar1=1.0 / 64, scalar2=None,
                                        op0=ALU.mult)
            pb = psD.tile([P, LQ], f32, tag="d", name="ps_b")
            nc.tensor.matmul(pb[:], lhsT=ones_row[:], rhs=pdc[:], start=True, stop=True)
            rb = finp.tile([P, LQ], f32, tag="rb")
            nc.vector.reciprocal_approx_fast(out=rb[:], in_=pb[:])
            ou = oup.tile([P, CS, LQ], fp8, tag="ou")
            for c_ in range(CS):
                nc.vector.tensor_tensor(out=ou[:, c_, :], in0=po[c_][:], in1=rb[:], op=ALU.mult)
            residual_pass(lt)
            fin_ps, fin_tag = (psA, "po") if lt < NLT - 1 else (psS, "s")

            for co_s in range(CS):
                pz = fin_ps.tile([P, LQ], f32, tag=fin_tag, name="ps_z")
                for cp in range(2):
                    nc.tensor.matmul(pz[:], lhsT=wsb["wp"][:, 2 * cp:2 * cp + 2, ts(co_s, P)],
                                     rhs=ou[:, 2 * cp:2 * cp + 2, :],
                                     start=(cp == 0), stop=(cp == 1), perf_mode=DR)
                fin = finp.tile([P, LQ], f32, tag="fin")
                nc.vector.scalar_tensor_tensor(out=fin[:], in0=pz[:], scalar=1.0 / 512,
                                               in1=xt[:, co_s, ts(lt, LQ)],
                                               op0=ALU.mult, op1=ALU.add)
                nc.sync.dma_start(out=out_dv[:, co_s, ts(lt, LQ)], in_=fin[:])

    nc.compile()
    return nc


def get_nc():
    if "nc" not in _CACHE:
        _CACHE["nc"] = _build_nc()
    return _CACHE["nc"]


def _g0_const():
    g = np.zeros((P, 2), np.float32)
    g[0:CPG, 0] = 1.0 / CPG
    g[CPG:P, 1] = 1.0 / CPG
    return g


def _sel_const():
    s = np.zeros((2, P), np.float32)
    s[0, 0:CPG] = 1.0
    s[1, CPG:P] = 1.0
    return s


def prep_inputs(x, gamma, beta, wq, bq, wk, bk, wv, bv, wp, bp):
    """Host-side layout prep (transposes / reshapes / bf16 weight casts, plus
    folding the 1/sqrt(C) attention scale into wk/bk). Per-core input maps."""
    import ml_dtypes

    f = np.float32
    bf = ml_dtypes.bfloat16
    f8 = ml_dtypes.float8_e4m3fn
    x = np.asarray(x, f)
    scale = f(C) ** f(-0.5)

    def wprep(w, s):
        # x8 / x4 pre-scale keeps the ~N(0, 0.02) weights in fp8e4m3's normal
        # range; the kernel divides the factors back out (copy scale=1/8 for
        # q/k, 4*8=32 folded into the softmax denominators for v/p).
        w = np.asarray(w, f) * s
        return np.ascontiguousarray(w.reshape(CS, P, C).transpose(1, 0, 2)).astype(f8)

    def vprep(v):
        v = np.asarray(v, f)
        return np.ascontiguousarray(v.reshape(CS, P).T)

    shared = {
        "wq": wprep(wq, 8), "wk": wprep(np.asarray(wk, f) * scale, 8),
        "wv": wprep(wv, 1), "wp": wprep(wp, 8),
        "vp": np.ascontiguousarray(np.concatenate(
            [vprep(gamma), vprep(beta), vprep(bq),
             vprep(np.asarray(bk, f) * scale), vprep(bp)], axis=1)),
        "bv_bcast": np.ascontiguousarray(
            np.broadcast_to(np.asarray(bv, f), (P, C))),
        "g0": _g0_const(), "sel": _sel_const(),
    }
    in_maps = []
    for b in range(N_CORES):
        m = dict(shared)
        xtb = np.ascontiguousarray(x[b].T)                       # [C, L]
        m["xb"] = xtb.astype(bf)
        # [NLT, P, CS, LQ]: per-lq-tile chunks of x^T in [p, s, j] layout
        m["xr"] = np.ascontiguousarray(
            xtb.reshape(CS, P, NLT, LQ).transpose(2, 1, 0, 3))
        in_maps.append(m)
    return in_maps


def run(inputs, trace=False, **kw):
    from concourse.bass_utils import run_bass_kernel_spmd

    nc = get_nc()
    in_maps = prep_inputs(**inputs)
    return run_bass_kernel_spmd(nc, in_maps, core_ids=list(range(N_CORES)),
                                trace=trace, **kw)


def kernel(**inputs) -> np.ndarray:
    res = run(inputs)
    out = np.empty((B, L, C), np.float32)
    for b in range(N_CORES):
        out[b] = res.results[b]["out_t"].T
    return out



# revision 8
# speedup vs baseline: 1.0807x; 1.0807x over previous
"""Trainium2 Bass kernel for nn_AttentionBlock (B=8, L=2048, C=512, GroupNorm(8) +
single-head attention + residual), data-parallel over batch across 8 NeuronCores.

Self-contained: hardcodes shapes/sharding. kernel(**inputs) -> np.ndarray [B,L,C].

Two-matmul attention: the four projection weights collapse into two on the host
  W1 = wq @ wk^T / sqrt(C)     (S = h W1 h^T  -- q/k projections fused)
  W2 = wv @ wp                 (attn @ (h W2) -- v/output projections fused)
so the device computes, per core / batch element (channel-major h^T [C, L]):
  x^T bf16 --bn_stats/group-reduce--> a_c, b_c ; rstd via exp(-.5 ln(var+eps))
  hb  = a*x + b                  (fp8, matmul operand; also the S lhsT = "K")
  hbf = a*x + b + (bp + bv@wp)   (bf16, residual + folded biases)
  U^T = W1^T h^T  (fp8)  ;  V' = h W2  (fp8, natural [L, C] layout)
  per 512-wide lq tile:
     for each 128-key block kb: S^T = hb-chunk^T @ U^T (PSUM); P = exp(S^T) fp8
     po += V'-chunk^T @ P  (PSUM accum) ; denom += 1^T @ P
     out^T = po * (1/denom) + hbf       (bf16, DMA'd out)
Per-query bias terms cancel in softmax (exact); per-key terms (only if bq != 0)
ride the exp's per-partition bias. All activation funcs live in one ACT table
set (natural_log_exp_and_others) so there is a single table load at t=0.
"""

import numpy as np

B, L, C = 8, 2048, 512
GROUPS = 8
EPS = 1e-3
P = 128
CS = C // P            # 4 channel subtiles of 128
LQ = 512               # lq tile width (matmul free dim)
NLT = L // LQ          # 4 lq tiles
NLB = L // P           # 16 key/l blocks
CPG = C // GROUPS      # 64 channels per group
N_CORES = 8

_CACHE = {}


def _build_nc(with_kappa):
    from contextlib import ExitStack

    import concourse.bass as bass
    import concourse.mybir as mybir
    import concourse.tile as tile
    from concourse import bacc
    from concourse.bass import ts

    f32 = mybir.dt.float32
    bf16 = mybir.dt.bfloat16
    fp8 = mybir.dt.float8e4
    DR = mybir.MatmulPerfMode.DoubleRow
    AF = mybir.ActivationFunctionType
    ALU = mybir.AluOpType

    nc = bacc.Bacc(trn_type="TRN2")

    xb_d = nc.dram_tensor("xb", [C, L], bf16, kind="ExternalInput")
    w_d = {
        n: nc.dram_tensor(n, [P, CS, C], fp8, kind="ExternalInput")
        for n in ("w1", "w2")
    }
    # packed per-channel vectors: [gamma, beta, bres] x CS columns
    vp_d = nc.dram_tensor("vp", [P, 3 * CS], f32, kind="ExternalInput")
    g0_d = nc.dram_tensor("g0", [P, 2], f32, kind="ExternalInput")
    sel_d = nc.dram_tensor("sel", [2, P], f32, kind="ExternalInput")
    if with_kappa:
        c3_d = nc.dram_tensor("c3", [P, CS, 1], fp8, kind="ExternalInput")
        ksc_d = nc.dram_tensor("ksc", [P, 2], f32, kind="ExternalInput")
    out_d = nc.dram_tensor("out_t", [C, L], bf16, kind="ExternalOutput")

    xb_dv = xb_d[:].rearrange("(s p) l -> p s l", p=P)
    out_dv = out_d[:].rearrange("(s p) l -> p s l", p=P)

    # scales (powers of two; host mirrors these exactly)
    KU = 5                 # ut = U * 2^KU
    KV = 5                 # vt = V' * 2^KV

    with tile.TileContext(nc) as tc, ExitStack() as ctx:
        consts = ctx.enter_context(tc.tile_pool(name="consts", bufs=1))
        data = ctx.enter_context(tc.tile_pool(name="data", bufs=1))
        small = ctx.enter_context(tc.tile_pool(name="small", bufs=1))
        ptp = ctx.enter_context(tc.tile_pool(name="ptp", bufs=3))
        t1p = ctx.enter_context(tc.tile_pool(name="t1p", bufs=2))
        outp = ctx.enter_context(tc.tile_pool(name="outp", bufs=4))
        finp = ctx.enter_context(tc.tile_pool(name="finp", bufs=2))
        psA = ctx.enter_context(tc.tile_pool(name="psA", bufs=4, space="PSUM"))
        psS = ctx.enter_context(tc.tile_pool(name="psS", bufs=3, space="PSUM"))
        psD = ctx.enter_context(tc.tile_pool(name="psD", bufs=1, space="PSUM"))

        # ---- SBUF residents ----
        xb = data.tile([P, CS, L], bf16)      # x^T bf16
        hb = data.tile([P, CS, L], fp8)       # h^T fp8 (matmul operand + S lhsT)
        hbf = data.tile([P, CS, L], bf16)     # h^T + bres (residual, bf16)
        ut = data.tile([P, CS, L], fp8)       # U^T * 2^KU
        vt = data.tile([P, NLB, C], fp8)      # V' natural, [l%P, l//P, c] * 2^KV
        wsb = {n: consts.tile([P, CS, C], fp8, name=f"w_{n}") for n in w_d}
        vp = consts.tile([P, 3 * CS], f32)
        GAM, BET, BRES = (vp[:, i * CS:(i + 1) * CS] for i in range(3))
        g0 = consts.tile([P, 2], f32)
        sel = consts.tile([2, P], f32)
        ones_col = consts.tile([P, 2, 16], fp8)   # [:, :, 0:1] = DoubleRow ones
        ones_row = consts.tile([1, P], bf16)
        eps2 = consts.tile([2, 1], f32)
        if with_kappa:
            c3v = consts.tile([P, CS, 1], fp8)
            kscv = consts.tile([P, 2], f32)
            ksb = small.tile([P, NLB], f32)

        # ---- loads + constants ----
        # Small/latency-critical first; xb subtiles split across four engine
        # queues so the stats pipeline starts ~3x sooner than one queue.
        nc.gpsimd.dma_start(out=g0[:], in_=g0_d[:])
        nc.gpsimd.dma_start(out=sel[:], in_=sel_d[:])
        nc.gpsimd.dma_start(out=vp[:], in_=vp_d[:])
        if with_kappa:
            nc.gpsimd.dma_start(out=c3v[:], in_=c3_d[:])
            nc.gpsimd.dma_start(out=kscv[:], in_=ksc_d[:])
        qeng = [nc.sync, nc.scalar, nc.gpsimd, nc.sync]
        for s in range(CS):
            qeng[s].dma_start(out=xb[:, s, :], in_=xb_dv[:, s, :])
        nc.scalar.dma_start(out=wsb["w1"][:], in_=w_d["w1"][:])
        nc.scalar.dma_start(out=wsb["w2"][:], in_=w_d["w2"][:])
        nc.vector.memset(ones_col[:], 1.0)
        nc.vector.memset(ones_row[:], 1.0)
        nc.vector.memset(eps2[:], EPS)
        # warm the (single) ACT table set immediately: Exp is the anchor of
        # natural_log_exp_and_others, which also holds Ln/Square/Identity.
        dm = small.tile([2, 1], f32, name="dm")
        nc.scalar.activation(out=dm[:], in_=eps2[:], func=AF.Exp)

        # ---- GroupNorm stats ----
        # per-channel (partition) sum / sumsq over L: subtiles 0,1 via DVE
        # bn_stats; subtiles 2,3 via ACT activation accum (Identity / Square),
        # chunked 512-wide so work starts as each DMA chunk lands.
        st = small.tile([P, CS, 2], f32)      # (mean_c, E[x^2]_c) per subtile
        for s in range(2):
            st6 = small.tile([P, 4, 6], f32, tag="st6", bufs=2)
            for j in range(4):
                nc.vector.bn_stats(out=st6[:, j, :], in_=xb[:, s, ts(j, 512)])
            mv = small.tile([P, 2], f32, tag="mv", bufs=2)
            nc.vector.bn_aggr(out=mv[:], in_=st6[:])
            nc.vector.tensor_copy(out=st[:, s, 0:1], in_=mv[:, 0:1])
            nc.vector.tensor_tensor(out=st[:, s, 1:2], in0=mv[:, 0:1], in1=mv[:, 0:1], op=ALU.mult)
            nc.vector.tensor_tensor(out=st[:, s, 1:2], in0=st[:, s, 1:2], in1=mv[:, 1:2], op=ALU.add)
        gscr = small.tile([P, 512], bf16)
        acc = small.tile([P, 2, CS, 2], f32)  # [p, sub, chunk-part, (sum, sumsq)]
        for s in (2, 3):
            for j in range(4):
                nc.scalar.activation(out=gscr[:], in_=xb[:, s, ts(j, 512)], func=AF.Identity,
                                     accum_out=acc[:, s - 2, j, 0:1])
                nc.scalar.activation(out=gscr[:], in_=xb[:, s, ts(j, 512)], func=AF.Square,
                                     accum_out=acc[:, s - 2, j, 1:2])
        accv = acc[:].rearrange("p a b c -> p a (b c)")
        for s in (2, 3):
            # sum the 4 chunk-partials, then /L
            ss = small.tile([P, 2], f32, tag="ss", bufs=2)
            nc.vector.tensor_tensor(out=ss[:], in0=accv[:, s - 2, 0:2], in1=accv[:, s - 2, 2:4], op=ALU.add)
            nc.vector.tensor_tensor(out=ss[:], in0=ss[:], in1=accv[:, s - 2, 4:6], op=ALU.add)
            nc.vector.tensor_tensor(out=ss[:], in0=ss[:], in1=accv[:, s - 2, 6:8], op=ALU.add)
            nc.vector.tensor_scalar(out=st[:, s, :], in0=ss[:], scalar1=1.0 / L, scalar2=None,
                                    op0=ALU.mult)

        psg = psD.tile([2, 2 * CS], f32, tag="d")   # [group-half, (s, stat)]
        nc.tensor.matmul(psg[:], lhsT=g0[:], rhs=st[:].rearrange("p a b -> p (a b)"),
                         start=True, stop=True)
        pst = small.tile([2, 2 * CS], f32)
        nc.vector.tensor_copy(out=pst[:], in_=psg[:])
        pstv = pst[:].rearrange("p (s k) -> p s k", k=2)
        msq = small.tile([2, CS], f32)
        nc.vector.tensor_tensor(out=msq[:], in0=pstv[:, :, 0], in1=pstv[:, :, 0], op=ALU.mult)
        grp = small.tile([2, 2 * CS], f32)     # [:, :CS]=rstd_g, [:, CS:]=mean_g
        nc.vector.tensor_tensor(out=grp[:, 0:CS], in0=pstv[:, :, 1], in1=msq[:], op=ALU.subtract)
        # rstd = exp(-0.5 * ln(var + eps)) -- stays in the one table set
        nc.scalar.activation(out=grp[:, 0:CS], in_=grp[:, 0:CS], func=AF.Ln,
                             bias=eps2[:], scale=1.0)
        nc.vector.tensor_scalar(out=grp[:, 0:CS], in0=grp[:, 0:CS], scalar1=-0.5, scalar2=None,
                                op0=ALU.mult)
        nc.scalar.activation(out=grp[:, 0:CS], in_=grp[:, 0:CS], func=AF.Exp)
        nc.vector.tensor_copy(out=grp[:, CS:], in_=pstv[:, :, 0])

        psbc = psD.tile([P, 2 * CS], f32, tag="d")  # broadcast groups -> channels
        nc.tensor.matmul(psbc[:], lhsT=sel[:], rhs=grp[:], start=True, stop=True)
        ab = small.tile([P, 2 * CS], f32)      # [:, :CS]=a_c, [:, CS:]=b_c
        nc.vector.tensor_tensor(out=ab[:, 0:CS], in0=GAM, in1=psbc[:, 0:CS], op=ALU.mult)
        nc.vector.tensor_tensor(out=ab[:, CS:], in0=psbc[:, CS:], in1=ab[:, 0:CS], op=ALU.mult)
        nc.vector.tensor_tensor(out=ab[:, CS:], in0=BET, in1=ab[:, CS:], op=ALU.subtract)
        # residual-pass intercept: b + bres (bres = bp + bv @ wp, host-folded)
        ab2 = small.tile([P, CS], f32)
        nc.vector.tensor_tensor(out=ab2[:], in0=ab[:, CS:], in1=BRES, op=ALU.add)

        # ---- normalize ----
        # hb (fp8 matmul operand) per (subtile, lt-slice) so the first U/V'
        # matmuls start after 4 small ops; split DVE/ACT. hbf (bf16 residual)
        # only feeds finales: lt0 early on DVE (2x mode), lt1-3 on GpSimd.
        def hb_slice(s, lt):
            if (s + lt) % 2 == 0:
                nc.vector.tensor_scalar(out=hb[:, s, ts(lt, LQ)], in0=xb[:, s, ts(lt, LQ)],
                                        scalar1=ab[:, s:s + 1], scalar2=ab[:, CS + s:CS + s + 1],
                                        op0=ALU.mult, op1=ALU.add)
            else:
                nc.scalar.activation(out=hb[:, s, ts(lt, LQ)], in_=xb[:, s, ts(lt, LQ)],
                                     func=AF.Identity,
                                     bias=ab[:, CS + s:CS + s + 1], scale=ab[:, s:s + 1])

        for lt in range(NLT):
            for s in range(CS):
                hb_slice(s, lt)
        for s in range(CS):
            nc.vector.tensor_scalar(out=hbf[:, s, ts(0, LQ)], in0=xb[:, s, ts(0, LQ)],
                                    scalar1=ab[:, s:s + 1], scalar2=ab2[:, s:s + 1],
                                    op0=ALU.mult, op1=ALU.add)
        for lt in range(1, NLT):
            for s in range(CS):
                nc.gpsimd.tensor_scalar(out=hbf[:, s, ts(lt, LQ)], in0=xb[:, s, ts(lt, LQ)],
                                        scalar1=ab[:, s:s + 1], scalar2=ab2[:, s:s + 1],
                                        op0=ALU.mult, op1=ALU.add)

        # ---- projections ----
        epi_ix = [0]

        def epi(dst, src, scl, eng):
            # PSUM -> SBUF fp8 cast with scale. 'alt' alternates DVE / ACT
            # (pre-attention, both engines have slack); 'dve' keeps the ACT
            # queue free for the exp stream during attention.
            if eng == "alt":
                eng = "dve" if epi_ix[0] % 2 == 0 else "act"
                epi_ix[0] += 1
            if eng == "dve":
                nc.vector.tensor_scalar(out=dst, in0=src, scalar1=scl, scalar2=None,
                                        op0=ALU.mult)
            else:
                nc.scalar.activation(out=dst, in_=src, func=AF.Identity, scale=scl)

        def u_proj(lt, eng="alt"):
            # ut[:, co_s, lq] = sum_ci W1[ci, co]^T h^T ; scale 2^KU / W1SC
            for co_s in range(CS):
                ps = psS.tile([P, LQ], f32, tag="s", name="ps_u")
                for cp in range(2):
                    nc.tensor.matmul(ps[:], lhsT=wsb["w1"][:, 2 * cp:2 * cp + 2, ts(co_s, P)],
                                     rhs=hb[:, 2 * cp:2 * cp + 2, ts(lt, LQ)],
                                     start=(cp == 0), stop=(cp == 1), perf_mode=DR)
                epi(ut[:, co_s, ts(lt, LQ)], ps[:], float(2.0 ** KU) / W1SC, eng)

        def v_proj(lb, eng="alt"):
            # vt[l%P, lb, :] = (h W2)*2^KV rows for key block lb
            ps = psS.tile([P, C], f32, tag="s", name="ps_v")
            for cp in range(2):
                nc.tensor.matmul(ps[:], lhsT=hb[:, 2 * cp:2 * cp + 2, ts(lb, P)],
                                 rhs=wsb["w2"][:, 2 * cp:2 * cp + 2, :],
                                 start=(cp == 0), stop=(cp == 1), perf_mode=DR)
            epi(vt[:, lb, :], ps[:], float(2.0 ** KV) / W2SC, eng)

        if with_kappa:
            # kappa[m] = h_m . c3 + const  (per-key exp bias; only if bq != 0)
            psk = psS.tile([P, NLB], f32, tag="s", name="ps_k")
            for kb in range(NLB):
                for cp in range(2):
                    nc.tensor.matmul(psk[:, kb:kb + 1],
                                     lhsT=hb[:, 2 * cp:2 * cp + 2, ts(kb, P)],
                                     rhs=c3v[:, 2 * cp:2 * cp + 2, :],
                                     start=(cp == 0), stop=(cp == 1), perf_mode=DR)
            nc.vector.tensor_scalar(out=ksb[:], in0=psk[:], scalar1=kscv[:, 0:1],
                                    scalar2=kscv[:, 1:2], op0=ALU.mult, op1=ALU.add)

        u_proj(0)
        for lb in range(4):
            v_proj(lb)

        # ---- attention, per lq tile ----
        EXPS = float(2.0 ** -KU)

        def finale(lt, po, pd):
            # pdc = denom * 2^KV (bf16) ; pb = bcast ; rb = 1 / pb ;
            # out = po * rb + hbf  (two DVE ops per 128-channel chunk)
            pdc = small.tile([1, LQ], bf16, tag="pdc", bufs=2)
            with nc.allow_low_precision(reason="denom rounded to bf16 as matmul operand"):
                nc.vector.tensor_scalar(out=pdc[:], in0=pd[:], scalar1=float(2.0 ** KV),
                                        scalar2=None, op0=ALU.mult)
            pb = psS.tile([P, LQ], f32, tag="s", name="ps_b")
            nc.tensor.matmul(pb[:], lhsT=ones_row[:], rhs=pdc[:], start=True, stop=True)
            rb = finp.tile([P, LQ], f32, tag="rb")
            nc.vector.reciprocal_approx_fast(out=rb[:], in_=pb[:])
            for c_ in range(CS):
                t1 = t1p.tile([P, LQ], bf16, tag="t1")
                with nc.allow_low_precision(reason="attn term to bf16"):
                    nc.vector.tensor_tensor(out=t1[:], in0=po[c_][:], in1=rb[:], op=ALU.mult)
                ot = outp.tile([P, LQ], bf16, tag="ot")
                nc.vector.tensor_tensor(out=ot[:], in0=t1[:], in1=hbf[:, c_, ts(lt, LQ)],
                                        op=ALU.add)
                # sync queue only: a dependent dma_start on the ACT/DVE queue
                # would stall that engine's instruction stream at the wait
                nc.sync.dma_start(out=out_dv[:, c_, ts(lt, LQ)], in_=ot[:])

        pending = None  # (lt, po, pd) awaiting finale emission
        for lt in range(NLT):
            po = [psA.tile([P, LQ], f32, tag="po", name=f"po{i}") for i in range(CS)]
            pd = psD.tile([1, LQ], f32, tag="d", name="pd")

            def pv_group(kp, pt2):
                for c_ in range(CS):
                    nc.tensor.matmul(po[c_][:], lhsT=vt[:, 2 * kp:2 * kp + 2, ts(c_, P)],
                                     rhs=pt2[:], start=(kp == 0), stop=(kp == NLB // 2 - 1),
                                     perf_mode=DR)
                nc.tensor.matmul(pd[:], lhsT=ones_col[:, :, 0:1], rhs=pt2[:],
                                 start=(kp == 0), stop=(kp == NLB // 2 - 1), perf_mode=DR)

            prev = None
            for kp in range(NLB // 2):
                pt2 = ptp.tile([P, 2, LQ], fp8, tag="pt")
                for i in range(2):
                    kb = 2 * kp + i
                    ps = psS.tile([P, LQ], f32, tag="s", name="ps_s")
                    for cp in range(2):
                        nc.tensor.matmul(ps[:], lhsT=hb[:, 2 * cp:2 * cp + 2, ts(kb, P)],
                                         rhs=ut[:, 2 * cp:2 * cp + 2, ts(lt, LQ)],
                                         start=(cp == 0), stop=(cp == 1), perf_mode=DR)
                    if with_kappa:
                        nc.scalar.activation(out=pt2[:, i, :], in_=ps[:], func=AF.Exp,
                                             scale=EXPS, bias=ksb[:, kb:kb + 1])
                    else:
                        nc.scalar.activation(out=pt2[:, i, :], in_=ps[:], func=AF.Exp,
                                             scale=EXPS)
                if kp == 0 and pending is not None:
                    # previous tile's finale: emitted after this tile's first
                    # S pair so the PE has queued work during the DVE chain
                    finale(*pending)
                    pending = None
                if kp == 1 and lt < NLT - 1:
                    # next tile's U slice streamed into this tile's PE slack
                    u_proj(lt + 1, eng="dve")
                if prev is not None:
                    pv_group(*prev)
                prev = (kp, pt2)
                if lt == 0 and kp < 6:
                    # V' blocks 4..15 interleaved two per kp; PV(kp) only
                    # needs blocks 2kp..2kp+1, produced >= 2 kps ahead
                    v_proj(4 + 2 * kp, eng="dve")
                    v_proj(5 + 2 * kp, eng="dve")
            pv_group(*prev)
            pending = (lt, po, pd)
        finale(*pending)

    nc.compile()
    return nc


def get_nc(with_kappa):
    key = ("nc", with_kappa, W1SC, W2SC)
    if key not in _CACHE:
        _CACHE[key] = _build_nc(with_kappa)
    return _CACHE[key]


def _g0_const():
    g = np.zeros((P, 2), np.float32)
    g[0:CPG, 0] = 1.0 / CPG
    g[CPG:P, 1] = 1.0 / CPG
    return g


def _sel_const():
    s = np.zeros((2, P), np.float32)
    s[0, 0:CPG] = 1.0
    s[1, CPG:P] = 1.0
    return s


def _pow2_scale(w, target=240.0):
    """Largest power-of-2 s with max|w|*s <= target (fp8e4m3 headroom)."""
    m = float(np.abs(w).max())
    if m == 0.0:
        return 1.0
    return float(2.0 ** np.floor(np.log2(target / m)))


# module-level so _build_nc sees the host-chosen weight scales
W1SC = 1.0
W2SC = 1.0


def prep_inputs(x, gamma, beta, wq, bq, wk, bk, wv, bv, wp, bp):
    """Host-side prep: fold wq@wk^T and wv@wp, transpose/cast, per-core maps."""
    global W1SC, W2SC
    import ml_dtypes

    f = np.float32
    bf = ml_dtypes.bfloat16
    f8 = ml_dtypes.float8_e4m3fn
    x = np.asarray(x, f)
    lam = f(C) ** f(-0.5)

    w1 = (np.asarray(wq, f) @ np.asarray(wk, f).T) * lam      # S = h w1 h^T
    w2 = np.asarray(wv, f) @ np.asarray(wp, f)                # o' = h w2
    bres = np.asarray(bp, f) + np.asarray(bv, f) @ np.asarray(wp, f)
    W1SC = _pow2_scale(w1)
    W2SC = _pow2_scale(w2)
    with_kappa = bool(np.any(np.asarray(bq, f)))

    def wprep(w, s):
        w = np.asarray(w, f) * s
        return np.ascontiguousarray(w.reshape(CS, P, C).transpose(1, 0, 2)).astype(f8)

    def vprep(v):
        v = np.asarray(v, f)
        return np.ascontiguousarray(v.reshape(CS, P).T)

    shared = {
        "w1": wprep(w1, W1SC), "w2": wprep(w2, W2SC),
        "vp": np.ascontiguousarray(np.concatenate(
            [vprep(gamma), vprep(beta), vprep(bres)], axis=1)),
        "g0": _g0_const(), "sel": _sel_const(),
    }
    if with_kappa:
        # kappa[m] = lam * (h_m wk) . bq + lam * bq.bk  (per-key exp bias)
        c3 = lam * (np.asarray(wk, f) @ np.asarray(bq, f))    # [C]
        c3s = _pow2_scale(c3)
        shared["c3"] = np.ascontiguousarray(
            (c3 * c3s).reshape(CS, P).T.reshape(P, CS, 1)).astype(f8)
        kconst = lam * float(np.asarray(bq, f) @ np.asarray(bk, f))
        ksc = np.empty((P, 2), f)
        ksc[:, 0] = 1.0 / c3s
        ksc[:, 1] = kconst
        shared["ksc"] = ksc
    in_maps = []
    for b in range(N_CORES):
        m = dict(shared)
        m["xb"] = np.ascontiguousarray(x[b].T).astype(bf)     # [C, L]
        in_maps.append(m)
    return in_maps, with_kappa


def run(inputs, trace=False, **kw):
    from concourse.bass_utils import run_bass_kernel_spmd

    in_maps, with_kappa = prep_inputs(**inputs)
    nc = get_nc(with_kappa)
    return run_bass_kernel_spmd(nc, in_maps, core_ids=list(range(N_CORES)),
                                trace=trace, **kw)


def kernel(**inputs) -> np.ndarray:
    res = run(inputs)
    out = np.empty((B, L, C), np.float32)
    for b in range(N_CORES):
        out[b] = np.asarray(res.results[b]["out_t"], np.float32).T
    return out


# revision 16
# speedup vs baseline: 1.1559x; 1.0695x over previous
"""Trainium2 Bass kernel for nn_AttentionBlock (B=8, L=2048, C=512, GroupNorm(8) +
single-head attention + residual), data-parallel over batch across 8 NeuronCores.

Self-contained: hardcodes shapes/sharding. kernel(**inputs) -> np.ndarray [B,L,C].

Two-matmul attention: the four projection weights collapse into two on the host
  W1 = wq @ wk^T / sqrt(C)     (S = h W1 h^T  -- q/k projections fused)
  W2 = wv @ wp                 (attn @ (h W2) -- v/output projections fused)
so the device computes, per core / batch element (channel-major h^T [C, L]):
  x^T fp16 --bn_stats/group-reduce--> a_c, b_c  (rstd via Newton-rsqrt on DVE,
    so the single ACT table set loaded at t=0 serves every activation)
  hb  = a*x + b                  (fp8, matmul operand; also the S lhsT = "K")
  hbf = a*x + b + (bp + bv@wp)   (fp16, residual + folded biases)
  U^T = W1^T h^T  (fp8)  ;  V' = h W2  (fp8, natural [L, C] layout)
  per 512-wide lq tile:
     for each 128-key block kb: S^T = hb-chunk^T @ U^T (PSUM); P = exp(S^T) fp8
     po += V'-chunk^T @ P  (PSUM accum) ; pd += (2^KV 1)^T @ P
     out^T = po * bcast(1/pd) + hbf     (fp16, DMA'd out)
Per-query bias terms cancel in softmax (exact); per-key terms (only if bq != 0)
ride the exp's per-partition bias.
"""

import numpy as np

B, L, C = 8, 2048, 512
GROUPS = 8
EPS = 1e-3
P = 128
CS = C // P            # 4 channel subtiles of 128
LQ = 512               # lq tile width (matmul free dim)
NLT = L // LQ          # 4 lq tiles
NLB = L // P           # 16 key/l blocks
CPG = C // GROUPS      # 64 channels per group
N_CORES = 8

_CACHE = {}


def _build_nc(with_kappa):
    from contextlib import ExitStack

    import concourse.bass as bass
    import concourse.mybir as mybir
    import concourse.tile as tile
    from concourse import bacc
    from concourse.bass import ts

    f32 = mybir.dt.float32
    f32r = mybir.dt.float32r
    fp16 = mybir.dt.float16
    bf16 = mybir.dt.bfloat16
    i32 = mybir.dt.int32
    fp8 = mybir.dt.float8e4
    DR = mybir.MatmulPerfMode.DoubleRow
    AF = mybir.ActivationFunctionType
    ALU = mybir.AluOpType

    nc = bacc.Bacc(trn_type="TRN2")

    xh_d = nc.dram_tensor("xh", [C, L], fp16, kind="ExternalInput")
    w_d = {
        n: nc.dram_tensor(n, [P, CS, C], fp8, kind="ExternalInput")
        for n in ("w1", "w2")
    }
    # packed per-channel vectors: [gamma, beta, bres] x CS columns
    vp_d = nc.dram_tensor("vp", [P, 3 * CS], f32, kind="ExternalInput")
    g0_d = nc.dram_tensor("g0", [P, 2], f32, kind="ExternalInput")
    sel_d = nc.dram_tensor("sel", [2, P], f32, kind="ExternalInput")
    mg_d = nc.dram_tensor("mg", [2, CS], i32, kind="ExternalInput")
    if with_kappa:
        c3_d = nc.dram_tensor("c3", [P, CS, 1], fp8, kind="ExternalInput")
        ksc_d = nc.dram_tensor("ksc", [P, 2], f32, kind="ExternalInput")
    out_d = nc.dram_tensor("out_t", [C, L], fp16, kind="ExternalOutput")

    xh_dv = xh_d[:].rearrange("(s p) l -> p s l", p=P)
    out_dv = out_d[:].rearrange("(s p) l -> p s l", p=P)

    # scales (powers of two; host mirrors these exactly)
    KU = 5                 # ut = U * 2^KU
    KV = 2                 # vt = V' * 2^KV ; denom-ones = 2^KV so po/pd cancels

    with tile.TileContext(nc) as tc, ExitStack() as ctx:
        consts = ctx.enter_context(tc.tile_pool(name="consts", bufs=1))
        data = ctx.enter_context(tc.tile_pool(name="data", bufs=1))
        small = ctx.enter_context(tc.tile_pool(name="small", bufs=1))
        ptp = ctx.enter_context(tc.tile_pool(name="ptp", bufs=3))
        t1p = ctx.enter_context(tc.tile_pool(name="t1p", bufs=2))
        outp = ctx.enter_context(tc.tile_pool(name="outp", bufs=4))
        finp = ctx.enter_context(tc.tile_pool(name="finp", bufs=2))
        psA = ctx.enter_context(tc.tile_pool(name="psA", bufs=4, space="PSUM"))
        psS = ctx.enter_context(tc.tile_pool(name="psS", bufs=3, space="PSUM"))
        psD = ctx.enter_context(tc.tile_pool(name="psD", bufs=1, space="PSUM"))

        # ---- SBUF residents ----
        xh = data.tile([P, CS, L], fp16)      # x^T fp16
        hb = data.tile([P, CS, L], fp8)       # h^T fp8 (matmul operand + S lhsT)
        hbf = data.tile([P, CS, L], fp16)     # h^T + bres (residual, fp16)
        ut = data.tile([P, CS, L], fp8)       # U^T * 2^KU
        vt = data.tile([P, NLB, C], fp8)      # V' natural, [l%P, l//P, c] * 2^KV
        wsb = {n: consts.tile([P, CS, C], fp8, name=f"w_{n}") for n in w_d}
        vp = consts.tile([P, 3 * CS], f32)
        GAM, BET, BRES = (vp[:, i * CS:(i + 1) * CS] for i in range(3))
        g0 = consts.tile([P, 2], f32)
        sel = consts.tile([2, P], f32)
        ones_col = consts.tile([P, 2, 16], fp8)   # [:, :, 0:1] = 2^KV (DR pair)
        ones_row = consts.tile([1, P], bf16)
        eps2 = consts.tile([2, 1], f32)
        magic = consts.tile([2, CS], i32)
        if with_kappa:
            c3v = consts.tile([P, CS, 1], fp8)
            kscv = consts.tile([P, 2], f32)
            ksb = small.tile([P, NLB], f32)

        # ---- loads + constants ----
        # 512-wide chunks round-robined over the three DMA-capable queues so
        # bn_stats starts on the first chunk ~1us after the preamble.
        nc.gpsimd.dma_start(out=g0[:], in_=g0_d[:])
        nc.gpsimd.dma_start(out=sel[:], in_=sel_d[:])
        nc.gpsimd.dma_start(out=vp[:], in_=vp_d[:])
        nc.gpsimd.dma_start(out=magic[:], in_=mg_d[:])
        if with_kappa:
            nc.gpsimd.dma_start(out=c3v[:], in_=c3_d[:])
            nc.gpsimd.dma_start(out=kscv[:], in_=ksc_d[:])
        for j in range(4):
            nc.sync.dma_start(out=xh[:, 0, ts(j, 512)], in_=xh_dv[:, 0, ts(j, 512)])
            nc.scalar.dma_start(out=xh[:, 1, ts(j, 512)], in_=xh_dv[:, 1, ts(j, 512)])
            nc.gpsimd.dma_start(out=xh[:, 2, ts(j, 512)], in_=xh_dv[:, 2, ts(j, 512)])
        for j in range(2):
            nc.sync.dma_start(out=xh[:, 3, ts(j, 512)], in_=xh_dv[:, 3, ts(j, 512)])
            nc.gpsimd.dma_start(out=xh[:, 3, ts(2 + j, 512)], in_=xh_dv[:, 3, ts(2 + j, 512)])
        nc.scalar.dma_start(out=wsb["w1"][:], in_=w_d["w1"][:])
        nc.scalar.dma_start(out=wsb["w2"][:], in_=w_d["w2"][:])
        nc.vector.memset(ones_col[:], float(2.0 ** KV))
        nc.vector.memset(ones_row[:], 1.0)
        nc.vector.memset(eps2[:], EPS)
        # warm the single ACT table set (exp_and_others: exp/identity/square)
        dm = small.tile([2, 1], f32, name="dm")
        nc.scalar.activation(out=dm[:], in_=eps2[:], func=AF.Exp)

        # ---- GroupNorm stats ----
        # per-channel (partition) sum / sumsq over L: subtiles 0-2 via DVE
        # bn_stats (chunk order matches DMA arrival); subtile 3 via ACT
        # activation accum (Identity / Square) on 1024-wide halves.
        st = small.tile([P, CS, 2], f32)      # (mean_c, E[x^2]_c) per subtile
        st6 = small.tile([P, 3, 4, 6], f32)
        for j in range(4):
            for s in range(3):
                nc.vector.bn_stats(out=st6[:, s, j, :], in_=xh[:, s, ts(j, 512)])
        gscr = small.tile([P, 1024], fp16)
        acc = small.tile([P, 2, 2], f32)      # [p, half, (sum, sumsq)]
        for h in range(2):
            nc.scalar.activation(out=gscr[:], in_=xh[:, 3, ts(h, 1024)], func=AF.Identity,
                                 accum_out=acc[:, h, 0:1])
            nc.scalar.activation(out=gscr[:], in_=xh[:, 3, ts(h, 1024)], func=AF.Square,
                                 accum_out=acc[:, h, 1:2])
        for s in range(3):
            mv = small.tile([P, 2], f32, tag="mv", bufs=2)
            nc.vector.bn_aggr(out=mv[:], in_=st6[:, s, :, :])
            nc.vector.tensor_copy(out=st[:, s, 0:1], in_=mv[:, 0:1])
            nc.vector.tensor_tensor(out=st[:, s, 1:2], in0=mv[:, 0:1], in1=mv[:, 0:1], op=ALU.mult)
            nc.vector.tensor_tensor(out=st[:, s, 1:2], in0=st[:, s, 1:2], in1=mv[:, 1:2], op=ALU.add)
        ss = small.tile([P, 2], f32)
        nc.vector.tensor_tensor(out=ss[:], in0=acc[:, 0, :], in1=acc[:, 1, :], op=ALU.add)
        nc.vector.tensor_scalar(out=st[:, 3, :], in0=ss[:], scalar1=1.0 / L, scalar2=None,
                                op0=ALU.mult)

        psg = psD.tile([2, 2 * CS], f32, tag="d")   # [group-half, (s, stat)]
        nc.tensor.matmul(psg[:], lhsT=g0[:], rhs=st[:].rearrange("p a b -> p (a b)"),
                         start=True, stop=True)
        pst = small.tile([2, 2 * CS], f32)
        nc.vector.tensor_copy(out=pst[:], in_=psg[:])
        pstv = pst[:].rearrange("p (s k) -> p s k", k=2)
        msq = small.tile([2, CS], f32)
        nc.vector.tensor_tensor(out=msq[:], in0=pstv[:, :, 0], in1=pstv[:, :, 0], op=ALU.mult)
        grp = small.tile([2, 2 * CS], f32)     # [:, :CS]=rstd_g, [:, CS:]=mean_g
        vv = small.tile([2, CS], f32)          # var + eps
        nc.vector.tensor_tensor(out=vv[:], in0=pstv[:, :, 1], in1=msq[:], op=ALU.subtract)
        nc.vector.tensor_scalar(out=vv[:], in0=vv[:], scalar1=1.0, scalar2=EPS,
                                op0=ALU.mult, op1=ALU.add)
        # rstd = rsqrt(var+eps): Quake bit-trick seed + 2 Newton iterations,
        # entirely on DVE -- no Sqrt/Ln table switches on the ACT engine.
        y = grp[:, 0:CS]
        yi = y.bitcast(i32)
        nc.vector.tensor_scalar(out=yi, in0=vv[:].bitcast(i32), scalar1=1, scalar2=None,
                                op0=ALU.logical_shift_right)
        nc.vector.tensor_tensor(out=yi, in0=magic[:], in1=yi, op=ALU.subtract)
        tn = small.tile([2, CS], f32)
        for _ in range(2):
            nc.vector.tensor_tensor(out=tn[:], in0=vv[:], in1=y, op=ALU.mult)
            nc.vector.tensor_tensor(out=tn[:], in0=tn[:], in1=y, op=ALU.mult)
            nc.vector.tensor_scalar(out=tn[:], in0=tn[:], scalar1=-0.5, scalar2=1.5,
                                    op0=ALU.mult, op1=ALU.add)
            nc.vector.tensor_tensor(out=y, in0=y, in1=tn[:], op=ALU.mult)
        nc.vector.tensor_copy(out=grp[:, CS:], in_=pstv[:, :, 0])

        psbc = psD.tile([P, 2 * CS], f32, tag="d")  # broadcast groups -> channels
        nc.tensor.matmul(psbc[:], lhsT=sel[:], rhs=grp[:], start=True, stop=True)
        ab = small.tile([P, 2 * CS], f32)      # [:, :CS]=a_c, [:, CS:]=b_c
        nc.vector.tensor_tensor(out=ab[:, 0:CS], in0=GAM, in1=psbc[:, 0:CS], op=ALU.mult)
        nc.vector.tensor_tensor(out=ab[:, CS:], in0=psbc[:, CS:], in1=ab[:, 0:CS], op=ALU.mult)
        nc.vector.tensor_tensor(out=ab[:, CS:], in0=BET, in1=ab[:, CS:], op=ALU.subtract)
        # residual-pass intercept: b + bres (bres = bp + bv @ wp, host-folded)
        ab2 = small.tile([P, CS], f32)
        nc.vector.tensor_tensor(out=ab2[:], in0=ab[:, CS:], in1=BRES, op=ALU.add)

        # ---- normalize ----
        # hb (fp8 matmul operand) per (subtile, lt-slice) so the first U/V'
        # matmuls start after 4 small ops; split DVE/ACT. hbf (fp16 residual)
        # only feeds finales: lt0 early on DVE, lt1-3 on GpSimd.
        def hb_slice(s, lt):
            if (s + lt) % 2 == 0:
                nc.vector.tensor_scalar(out=hb[:, s, ts(lt, LQ)], in0=xh[:, s, ts(lt, LQ)],
                                        scalar1=ab[:, s:s + 1], scalar2=ab[:, CS + s:CS + s + 1],
                                        op0=ALU.mult, op1=ALU.add)
            else:
                nc.scalar.activation(out=hb[:, s, ts(lt, LQ)], in_=xh[:, s, ts(lt, LQ)],
                                     func=AF.Identity,
                                     bias=ab[:, CS + s:CS + s + 1], scale=ab[:, s:s + 1])

        for lt in range(NLT):
            for s in range(CS):
                hb_slice(s, lt)
        for s in range(CS):
            nc.vector.tensor_scalar(out=hbf[:, s, ts(0, LQ)], in0=xh[:, s, ts(0, LQ)],
                                    scalar1=ab[:, s:s + 1], scalar2=ab2[:, s:s + 1],
                                    op0=ALU.mult, op1=ALU.add)
        for lt in range(1, NLT):
            for s in range(CS):
                nc.gpsimd.tensor_scalar(out=hbf[:, s, ts(lt, LQ)], in0=xh[:, s, ts(lt, LQ)],
                                        scalar1=ab[:, s:s + 1], scalar2=ab2[:, s:s + 1],
                                        op0=ALU.mult, op1=ALU.add)

        # ---- projections ----
        epi_ix = [0]

        def epi(dst, src, scl, eng):
            # PSUM -> SBUF fp8 cast with scale. 'alt' alternates DVE / ACT
            # (pre-attention, both engines have slack); 'dve' keeps the ACT
            # queue free for the exp stream during attention.
            if eng == "alt":
                eng = "dve" if epi_ix[0] % 2 == 0 else "act"
                epi_ix[0] += 1
            if eng == "dve":
                nc.vector.tensor_scalar(out=dst, in0=src, scalar1=scl, scalar2=None,
                                        op0=ALU.mult)
            else:
                nc.scalar.activation(out=dst, in_=src, func=AF.Identity, scale=scl)

        def u_proj(lt, eng="alt"):
            # ut[:, co_s, lq] = sum_ci W1[ci, co]^T h^T ; scale 2^KU / W1SC
            for co_s in range(CS):
                ps = psS.tile([P, LQ], f32, tag="s", name="ps_u")
                for cp in range(2):
                    nc.tensor.matmul(ps[:], lhsT=wsb["w1"][:, 2 * cp:2 * cp + 2, ts(co_s, P)],
                                     rhs=hb[:, 2 * cp:2 * cp + 2, ts(lt, LQ)],
                                     start=(cp == 0), stop=(cp == 1), perf_mode=DR)
                epi(ut[:, co_s, ts(lt, LQ)], ps[:], float(2.0 ** KU) / W1SC, eng)

        def v_proj(lb, eng="alt"):
            # vt[l%P, lb, :] = (h W2)*2^KV rows for key block lb
            ps = psS.tile([P, C], f32, tag="s", name="ps_v")
            for cp in range(2):
                nc.tensor.matmul(ps[:], lhsT=hb[:, 2 * cp:2 * cp + 2, ts(lb, P)],
                                 rhs=wsb["w2"][:, 2 * cp:2 * cp + 2, :],
                                 start=(cp == 0), stop=(cp == 1), perf_mode=DR)
            epi(vt[:, lb, :], ps[:], float(2.0 ** KV) / W2SC, eng)

        if with_kappa:
            # kappa[m] = h_m . c3 + const  (per-key exp bias; only if bq != 0)
            psk = psS.tile([P, NLB], f32, tag="s", name="ps_k")
            for kb in range(NLB):
                for cp in range(2):
                    nc.tensor.matmul(psk[:, kb:kb + 1],
                                     lhsT=hb[:, 2 * cp:2 * cp + 2, ts(kb, P)],
                                     rhs=c3v[:, 2 * cp:2 * cp + 2, :],
                                     start=(cp == 0), stop=(cp == 1), perf_mode=DR)
            nc.vector.tensor_scalar(out=ksb[:], in0=psk[:], scalar1=kscv[:, 0:1],
                                    scalar2=kscv[:, 1:2], op0=ALU.mult, op1=ALU.add)

        u_proj(0)
        for lb in range(4):
            v_proj(lb)

        # ---- attention, per lq tile ----
        EXPS = float(2.0 ** -KU)

        def finale(lt, po, pd):
            # pd = 2^KV * denom ; rb = bcast(1/pd) ; out = po*rb + hbf
            pdc = small.tile([1, LQ], bf16, tag="pdc", bufs=2)
            with nc.allow_low_precision(reason="denom rounded to bf16 as matmul operand"):
                nc.vector.tensor_scalar(out=pdc[:], in0=pd[:], scalar1=1.0, scalar2=None,
                                        op0=ALU.mult)
            pb = psS.tile([P, LQ], f32, tag="s", name="ps_b")
            nc.tensor.matmul(pb[:], lhsT=ones_row[:], rhs=pdc[:], start=True, stop=True)
            rb = finp.tile([P, LQ], f32, tag="rb")
            nc.vector.reciprocal_approx_fast(out=rb[:], in_=pb[:])
            for c_ in range(CS):
                t1 = t1p.tile([P, LQ], fp16, tag="t1")
                with nc.allow_low_precision(reason="attn term to fp16"):
                    nc.vector.tensor_tensor(out=t1[:], in0=po[c_][:], in1=rb[:], op=ALU.mult)
                ot = outp.tile([P, LQ], fp16, tag="ot")
                nc.vector.tensor_tensor(out=ot[:], in0=t1[:], in1=hbf[:, c_, ts(lt, LQ)],
                                        op=ALU.add)
                # sync queue only: a dependent dma_start on the ACT queue
                # would stall the exp instruction stream at the wait
                nc.sync.dma_start(out=out_dv[:, c_, ts(lt, LQ)], in_=ot[:])

        pending = None  # (lt, po, pd) awaiting finale emission
        for lt in range(NLT):
            po = [psA.tile([P, LQ], f32, tag="po", name=f"po{i}") for i in range(CS)]
            pd = psD.tile([1, LQ], f32, tag="d", name="pd")

            def pv_group(kp, pt2):
                for c_ in range(CS):
                    nc.tensor.matmul(po[c_][:], lhsT=vt[:, 2 * kp:2 * kp + 2, ts(c_, P)],
                                     rhs=pt2[:], start=(kp == 0), stop=(kp == NLB // 2 - 1),
                                     perf_mode=DR)
                nc.tensor.matmul(pd[:], lhsT=ones_col[:, :, 0:1], rhs=pt2[:],
                                 start=(kp == 0), stop=(kp == NLB // 2 - 1), perf_mode=DR)

            prev = None
            for kp in range(NLB // 2):
                pt2 = ptp.tile([P, 2, LQ], fp8, tag="pt")
                for i in range(2):
                    kb = 2 * kp + i
                    ps = psS.tile([P, LQ], f32, tag="s", name="ps_s")
                    for cp in range(2):
                        nc.tensor.matmul(ps[:], lhsT=hb[:, 2 * cp:2 * cp + 2, ts(kb, P)],
                                         rhs=ut[:, 2 * cp:2 * cp + 2, ts(lt, LQ)],
                                         start=(cp == 0), stop=(cp == 1), perf_mode=DR)
                    if with_kappa:
                        nc.scalar.activation(out=pt2[:, i, :], in_=ps[:], func=AF.Exp,
                                             scale=EXPS, bias=ksb[:, kb:kb + 1])
                    else:
                        nc.scalar.activation(out=pt2[:, i, :], in_=ps[:], func=AF.Exp,
                                             scale=EXPS)
                if kp == 0 and pending is not None:
                    # previous tile's finale: emitted after this tile's first
                    # S pair so the PE has queued work during the DVE chain
                    finale(*pending)
                    pending = None
                if kp == 1 and lt < NLT - 1:
                    # next tile's U slice streamed into this tile's PE slack
                    u_proj(lt + 1, eng="dve")
                if prev is not None:
                    pv_group(*prev)
                prev = (kp, pt2)
                if lt == 0 and kp < 6:
                    # V' blocks 4..15 interleaved two per kp; PV(kp) only
                    # needs blocks 2kp..2kp+1, produced >= 2 kps ahead
                    v_proj(4 + 2 * kp, eng="dve")
                    v_proj(5 + 2 * kp, eng="dve")
            pv_group(*prev)
            pending = (lt, po, pd)
        finale(*pending)

    nc.compile()
    return nc


def get_nc(with_kappa):
    key = ("nc", with_kappa, W1SC, W2SC)
    if key not in _CACHE:
        _CACHE[key] = _build_nc(with_kappa)
    return _CACHE[key]


def _g0_const():
    g = np.zeros((P, 2), np.float32)
    g[0:CPG, 0] = 1.0 / CPG
    g[CPG:P, 1] = 1.0 / CPG
    return g


def _sel_const():
    s = np.zeros((2, P), np.float32)
    s[0, 0:CPG] = 1.0
    s[1, CPG:P] = 1.0
    return s


def _pow2_scale(w, target=240.0):
    """Largest power-of-2 s with max|w|*s <= target (fp8e4m3 headroom)."""
    m = float(np.abs(w).max())
    if m == 0.0:
        return 1.0
    return float(2.0 ** np.floor(np.log2(target / m)))


# module-level so _build_nc sees the host-chosen weight scales
W1SC = 1.0
W2SC = 1.0


def prep_inputs(x, gamma, beta, wq, bq, wk, bk, wv, bv, wp, bp):
    """Host-side prep: fold wq@wk^T and wv@wp, transpose/cast, per-core maps."""
    global W1SC, W2SC
    import ml_dtypes

    f = np.float32
    f8 = ml_dtypes.float8_e4m3fn
    x = np.asarray(x, f)
    lam = f(C) ** f(-0.5)

    w1 = (np.asarray(wq, f) @ np.asarray(wk, f).T) * lam      # S = h w1 h^T
    w2 = np.asarray(wv, f) @ np.asarray(wp, f)                # o' = h w2
    bres = np.asarray(bp, f) + np.asarray(bv, f) @ np.asarray(wp, f)
    W1SC = _pow2_scale(w1)
    W2SC = _pow2_scale(w2)
    with_kappa = bool(np.any(np.asarray(bq, f)))

    def wprep(w, s):
        w = np.asarray(w, f) * s
        return np.ascontiguousarray(w.reshape(CS, P, C).transpose(1, 0, 2)).astype(f8)

    def vprep(v):
        v = np.asarray(v, f)
        return np.ascontiguousarray(v.reshape(CS, P).T)

    shared = {
        "w1": wprep(w1, W1SC), "w2": wprep(w2, W2SC),
        "vp": np.ascontiguousarray(np.concatenate(
            [vprep(gamma), vprep(beta), vprep(bres)], axis=1)),
        "g0": _g0_const(), "sel": _sel_const(),
        "mg": np.full((2, CS), 0x5F3759DF, np.int32),
    }
    if with_kappa:
        # kappa[m] = lam * (h_m wk) . bq + lam * bq.bk  (per-key exp bias)
        c3 = lam * (np.asarray(wk, f) @ np.asarray(bq, f))    # [C]
        c3s = _pow2_scale(c3)
        shared["c3"] = np.ascontiguousarray(
            (c3 * c3s).reshape(CS, P).T.reshape(P, CS, 1)).astype(f8)
        kconst = lam * float(np.asarray(bq, f) @ np.asarray(bk, f))
        ksc = np.empty((P, 2), f)
        ksc[:, 0] = 1.0 / c3s
        ksc[:, 1] = kconst
        shared["ksc"] = ksc
    in_maps = []
    for b in range(N_CORES):
        m = dict(shared)
        m["xh"] = np.ascontiguousarray(x[b].T).astype(np.float16)  # [C, L]
        in_maps.append(m)
    return in_maps, with_kappa


def run(inputs, trace=False, **kw):
    from concourse.bass_utils import run_bass_kernel_spmd

    in_maps, with_kappa = prep_inputs(**inputs)
    nc = get_nc(with_kappa)
    return run_bass_kernel_spmd(nc, in_maps, core_ids=list(range(N_CORES)),
                                trace=trace, **kw)


def kernel(**inputs) -> np.ndarray:
    res = run(inputs)
    out = np.empty((B, L, C), np.float32)
    for b in range(N_CORES):
        out[b] = np.asarray(res.results[b]["out_t"], np.float32).T
    return out


# revision 27
# speedup vs baseline: 1.2153x; 1.0514x over previous
"""Trainium2 Bass kernel for nn_AttentionBlock (B=8, L=2048, C=512, GroupNorm(8) +
single-head attention + residual), data-parallel over batch across 8 NeuronCores.

Self-contained: hardcodes shapes/sharding. kernel(**inputs) -> np.ndarray [B,L,C].

Two-matmul attention: the four projection weights collapse into two on the host
  W1 = wq @ wk^T / sqrt(C)     (S = h W1 h^T  -- q/k projections fused)
  W2 = wv @ wp                 (attn @ (h W2) -- v/output projections fused)
so the device computes, per core / batch element (channel-major h^T [C, L]):
  x^T fp16 --bn_stats/group-reduce--> a_c, b_c  (rstd via Newton-rsqrt on DVE,
    so the single ACT table set loaded at t=0 serves every activation)
  hb  = a*x + b                  (fp8, matmul operand; also the S lhsT = "K")
  hbf = a*x + b + (bp + bv@wp)   (fp16, residual + folded biases)
  U^T = W1^T h^T  (fp8)  ;  V' = h W2  (fp8, natural [L, C] layout)
  per 512-wide lq tile:
     for each 128-key block kb: S^T = hb-chunk^T @ U^T (PSUM); P = exp(S^T) fp8
     po += V'-chunk^T @ P  (PSUM accum) ; pd += (2^KV 1)^T @ P
     out^T = po * bcast(1/pd) + hbf     (fp16, DMA'd out)
Per-query bias terms cancel in softmax (exact); per-key terms (only if bq != 0)
ride the exp's per-partition bias.
"""

import numpy as np

B, L, C = 8, 2048, 512
GROUPS = 8
EPS = 1e-3
P = 128
CS = C // P            # 4 channel subtiles of 128
LQ = 512               # lq tile width (matmul free dim)
NLT = L // LQ          # 4 lq tiles
NLB = L // P           # 16 key/l blocks
CPG = C // GROUPS      # 64 channels per group
N_CORES = 8

_CACHE = {}


def _build_nc(with_kappa):
    from contextlib import ExitStack

    import concourse.bass as bass
    import concourse.mybir as mybir
    import concourse.tile as tile
    from concourse import bacc
    from concourse.bass import ts

    f32 = mybir.dt.float32
    f32r = mybir.dt.float32r
    fp16 = mybir.dt.float16
    bf16 = mybir.dt.bfloat16
    i32 = mybir.dt.int32
    fp8 = mybir.dt.float8e4
    DR = mybir.MatmulPerfMode.DoubleRow
    AF = mybir.ActivationFunctionType
    ALU = mybir.AluOpType

    nc = bacc.Bacc(trn_type="TRN2")

    # chunk-major [s*4+j, p, 512]: each 128KB DMA reads contiguous DRAM
    xh_d = nc.dram_tensor("xh", [4 * CS, P, 512], fp16, kind="ExternalInput")
    w_d = {
        n: nc.dram_tensor(n, [P, CS, C], fp8, kind="ExternalInput")
        for n in ("w1", "w2")
    }
    # packed per-channel vectors: [gamma, beta, bres] x CS columns
    vp_d = nc.dram_tensor("vp", [P, 3 * CS], f32, kind="ExternalInput")
    g0_d = nc.dram_tensor("g0", [P, 2], f32, kind="ExternalInput")
    sel_d = nc.dram_tensor("sel", [2, P], f32, kind="ExternalInput")
    mg_d = nc.dram_tensor("mg", [2, CS], i32, kind="ExternalInput")
    if with_kappa:
        c3_d = nc.dram_tensor("c3", [P, CS, 1], fp8, kind="ExternalInput")
        ksc_d = nc.dram_tensor("ksc", [P, 2], f32, kind="ExternalInput")
    out_d = nc.dram_tensor("out_t", [C, L], fp16, kind="ExternalOutput")

    out_dv = out_d[:].rearrange("(s p) l -> p s l", p=P)

    # scales (powers of two; host mirrors these exactly)
    KU = 5                 # ut = U * 2^KU
    KV = 2                 # vt = V' * 2^KV ; denom-ones = 2^KV so po/pd cancels

    with tile.TileContext(nc) as tc, ExitStack() as ctx:
        consts = ctx.enter_context(tc.tile_pool(name="consts", bufs=1))
        data = ctx.enter_context(tc.tile_pool(name="data", bufs=1))
        small = ctx.enter_context(tc.tile_pool(name="small", bufs=1))
        ptp = ctx.enter_context(tc.tile_pool(name="ptp", bufs=3))
        t1p = ctx.enter_context(tc.tile_pool(name="t1p", bufs=2))
        outp = ctx.enter_context(tc.tile_pool(name="outp", bufs=4))
        finp = ctx.enter_context(tc.tile_pool(name="finp", bufs=2))
        psA = ctx.enter_context(tc.tile_pool(name="psA", bufs=4, space="PSUM"))
        psS = ctx.enter_context(tc.tile_pool(name="psS", bufs=3, space="PSUM"))
        psD = ctx.enter_context(tc.tile_pool(name="psD", bufs=1, space="PSUM"))

        # ---- SBUF residents ----
        xh = data.tile([P, CS, L], fp16)      # x^T fp16
        hb = data.tile([P, CS, L], fp8)       # h^T fp8 (matmul operand + S lhsT)
        hbf = data.tile([P, CS, L], fp16)     # h^T + bres (residual, fp16)
        ut = data.tile([P, CS, L], fp8)       # U^T * 2^KU
        vt = data.tile([P, NLB, C], fp8)      # V' natural, [l%P, l//P, c] * 2^KV
        wsb = {n: consts.tile([P, CS, C], fp8, name=f"w_{n}") for n in w_d}
        vp = consts.tile([P, 3 * CS], f32)
        GAM, BET, BRES = (vp[:, i * CS:(i + 1) * CS] for i in range(3))
        g0 = consts.tile([P, 2], f32)
        sel = consts.tile([2, P], f32)
        ones_col = consts.tile([P, 2, 16], fp8)   # [:, :, 0:1] = 2^KV (DR pair)
        ones_row = consts.tile([1, P], bf16)
        eps2 = consts.tile([2, 1], f32)
        magic = consts.tile([2, CS], i32)
        if with_kappa:
            c3v = consts.tile([P, CS, 1], fp8)
            kscv = consts.tile([P, 2], f32)
            ksb = small.tile([P, NLB], f32)

        # ---- loads + constants ----
        # 512-wide chunks round-robined over the three DMA-capable queues so
        # bn_stats starts on the first chunk ~1us after the preamble.
        nc.gpsimd.dma_start(out=g0[:], in_=g0_d[:])
        nc.gpsimd.dma_start(out=sel[:], in_=sel_d[:])
        nc.gpsimd.dma_start(out=vp[:], in_=vp_d[:])
        nc.gpsimd.dma_start(out=magic[:], in_=mg_d[:])
        if with_kappa:
            nc.gpsimd.dma_start(out=c3v[:], in_=c3_d[:])
            nc.gpsimd.dma_start(out=kscv[:], in_=ksc_d[:])

        def xchunk(q, s, j):
            q.dma_start(out=xh[:, s, ts(j, 512)], in_=xh_d[4 * s + j])

        # per-queue order: s3's halves land first for the ACT accum path,
        # then the bn_stats subtiles in chunk order
        xchunk(nc.sync, 3, 0)
        xchunk(nc.scalar, 3, 1)
        for j in range(4):
            xchunk(nc.sync, 0, j)
            xchunk(nc.scalar, 1, j)
            xchunk(nc.gpsimd, 2, j)
            if j == 0:
                xchunk(nc.gpsimd, 3, 2)
                xchunk(nc.gpsimd, 3, 3)
        nc.scalar.dma_start(out=wsb["w1"][:], in_=w_d["w1"][:])
        nc.scalar.dma_start(out=wsb["w2"][:], in_=w_d["w2"][:])
        nc.vector.memset(ones_col[:], float(2.0 ** KV))
        nc.vector.memset(ones_row[:], 1.0)
        nc.vector.memset(eps2[:], EPS)
        # warm the single ACT table set (exp_and_others: exp/identity/square)
        dm = small.tile([2, 1], f32, name="dm")
        nc.scalar.activation(out=dm[:], in_=eps2[:], func=AF.Exp)

        # ---- GroupNorm stats ----
        # per-channel (partition) sum / sumsq over L: subtiles 0-2 via DVE
        # bn_stats (chunk order matches DMA arrival); subtile 3 via ACT
        # activation accum (Identity / Square) on 1024-wide halves.
        st = small.tile([P, CS, 2], f32)      # (mean_c, E[x^2]_c) per subtile
        st6 = small.tile([P, 3, 4, 6], f32)
        for j in range(4):
            for s in range(3):
                nc.vector.bn_stats(out=st6[:, s, j, :], in_=xh[:, s, ts(j, 512)])
        gscr = small.tile([P, 1024], fp16)
        acc = small.tile([P, 2, 2], f32)      # [p, half, (sum, sumsq)]
        for h in range(2):
            nc.scalar.activation(out=gscr[:], in_=xh[:, 3, ts(h, 1024)], func=AF.Identity,
                                 accum_out=acc[:, h, 0:1])
            nc.scalar.activation(out=gscr[:], in_=xh[:, 3, ts(h, 1024)], func=AF.Square,
                                 accum_out=acc[:, h, 1:2])
        for s in range(3):
            mv = small.tile([P, 2], f32, tag="mv", bufs=2)
            nc.vector.bn_aggr(out=mv[:], in_=st6[:, s, :, :])
            nc.vector.tensor_copy(out=st[:, s, 0:1], in_=mv[:, 0:1])
            nc.vector.tensor_tensor(out=st[:, s, 1:2], in0=mv[:, 0:1], in1=mv[:, 0:1], op=ALU.mult)
            nc.vector.tensor_tensor(out=st[:, s, 1:2], in0=st[:, s, 1:2], in1=mv[:, 1:2], op=ALU.add)
        ss = small.tile([P, 2], f32)
        nc.vector.tensor_tensor(out=ss[:], in0=acc[:, 0, :], in1=acc[:, 1, :], op=ALU.add)
        nc.vector.tensor_scalar(out=st[:, 3, :], in0=ss[:], scalar1=1.0 / L, scalar2=None,
                                op0=ALU.mult)

        psg = psD.tile([2, 2 * CS], f32, tag="d")   # [group-half, (s, stat)]
        nc.tensor.matmul(psg[:], lhsT=g0[:], rhs=st[:].rearrange("p a b -> p (a b)"),
                         start=True, stop=True)
        pst = small.tile([2, 2 * CS], f32)
        nc.vector.tensor_copy(out=pst[:], in_=psg[:])
        pstv = pst[:].rearrange("p (s k) -> p s k", k=2)
        msq = small.tile([2, CS], f32)
        nc.vector.tensor_tensor(out=msq[:], in0=pstv[:, :, 0], in1=pstv[:, :, 0], op=ALU.mult)
        grp = small.tile([2, 2 * CS], f32)     # [:, :CS]=rstd_g, [:, CS:]=mean_g
        vv = small.tile([2, CS], f32)          # var + eps
        nc.vector.tensor_tensor(out=vv[:], in0=pstv[:, :, 1], in1=msq[:], op=ALU.subtract)
        nc.vector.tensor_scalar(out=vv[:], in0=vv[:], scalar1=1.0, scalar2=EPS,
                                op0=ALU.mult, op1=ALU.add)
        # rstd = rsqrt(var+eps): Quake bit-trick seed + 2 Newton iterations,
        # entirely on DVE -- no Sqrt/Ln table switches on the ACT engine.
        y = grp[:, 0:CS]
        yi = y.bitcast(i32)
        nc.vector.tensor_scalar(out=yi, in0=vv[:].bitcast(i32), scalar1=1, scalar2=None,
                                op0=ALU.logical_shift_right)
        nc.vector.tensor_tensor(out=yi, in0=magic[:], in1=yi, op=ALU.subtract)
        tn = small.tile([2, CS], f32)
        for _ in range(2):
            nc.vector.tensor_tensor(out=tn[:], in0=vv[:], in1=y, op=ALU.mult)
            nc.vector.tensor_tensor(out=tn[:], in0=tn[:], in1=y, op=ALU.mult)
            nc.vector.tensor_scalar(out=tn[:], in0=tn[:], scalar1=-0.5, scalar2=1.5,
                                    op0=ALU.mult, op1=ALU.add)
            nc.vector.tensor_tensor(out=y, in0=y, in1=tn[:], op=ALU.mult)
        nc.vector.tensor_copy(out=grp[:, CS:], in_=pstv[:, :, 0])

        psbc = psD.tile([P, 2 * CS], f32, tag="d")  # broadcast groups -> channels
        nc.tensor.matmul(psbc[:], lhsT=sel[:], rhs=grp[:], start=True, stop=True)
        ab = small.tile([P, 2 * CS], f32)      # [:, :CS]=a_c, [:, CS:]=b_c
        nc.vector.tensor_tensor(out=ab[:, 0:CS], in0=GAM, in1=psbc[:, 0:CS], op=ALU.mult)
        nc.vector.tensor_tensor(out=ab[:, CS:], in0=psbc[:, CS:], in1=ab[:, 0:CS], op=ALU.mult)
        nc.vector.tensor_tensor(out=ab[:, CS:], in0=BET, in1=ab[:, CS:], op=ALU.subtract)
        # residual-pass intercept: b + bres (bres = bp + bv @ wp, host-folded)
        ab2 = small.tile([P, CS], f32)
        nc.vector.tensor_tensor(out=ab2[:], in0=ab[:, CS:], in1=BRES, op=ALU.add)

        # ---- normalize ----
        # hb (fp8 matmul operand) per (subtile, lt-slice) so the first U/V'
        # matmuls start after 4 small ops; split DVE/ACT. hbf (fp16 residual)
        # only feeds finales: lt0 early on DVE, lt1-3 on GpSimd.
        def hb_slice(s, lt):
            if (s + lt) % 2 == 0:
                nc.vector.tensor_scalar(out=hb[:, s, ts(lt, LQ)], in0=xh[:, s, ts(lt, LQ)],
                                        scalar1=ab[:, s:s + 1], scalar2=ab[:, CS + s:CS + s + 1],
                                        op0=ALU.mult, op1=ALU.add)
            else:
                nc.scalar.activation(out=hb[:, s, ts(lt, LQ)], in_=xh[:, s, ts(lt, LQ)],
                                     func=AF.Identity,
                                     bias=ab[:, CS + s:CS + s + 1], scale=ab[:, s:s + 1])

        # only lt0's hb slices gate the first U/V'/S matmuls; lt1/lt2 slices
        # are emitted inside lt0's kp loop (DVE+ACT slack), lt3 + all of hbf
        # on GpSimd (first needed at kp6 / the first finale respectively)
        for s in range(CS):
            hb_slice(s, 0)
        for lt in range(NLT):
            for s in range(CS):
                nc.gpsimd.tensor_scalar(out=hbf[:, s, ts(lt, LQ)], in0=xh[:, s, ts(lt, LQ)],
                                        scalar1=ab[:, s:s + 1], scalar2=ab2[:, s:s + 1],
                                        op0=ALU.mult, op1=ALU.add)

        # ---- projections ----
        epi_ix = [0]

        def epi(dst, src, scl, eng):
            # PSUM -> SBUF fp8 cast with scale. 'alt' alternates DVE / ACT
            # (pre-attention, both engines have slack); 'dve' keeps the ACT
            # queue free for the exp stream during attention.
            if eng == "alt":
                eng = "dve" if epi_ix[0] % 2 == 0 else "act"
                epi_ix[0] += 1
            if eng == "dve":
                nc.vector.tensor_scalar(out=dst, in0=src, scalar1=scl, scalar2=None,
                                        op0=ALU.mult)
            else:
                nc.scalar.activation(out=dst, in_=src, func=AF.Identity, scale=scl)

        def u_proj(lt, eng="alt"):
            # ut[:, co_s, lq] = sum_ci W1[ci, co]^T h^T ; scale 2^KU / W1SC
            for co_s in range(CS):
                ps = psS.tile([P, LQ], f32, tag="s", name="ps_u")
                for cp in range(2):
                    nc.tensor.matmul(ps[:], lhsT=wsb["w1"][:, 2 * cp:2 * cp + 2, ts(co_s, P)],
                                     rhs=hb[:, 2 * cp:2 * cp + 2, ts(lt, LQ)],
                                     start=(cp == 0), stop=(cp == 1), perf_mode=DR)
                epi(ut[:, co_s, ts(lt, LQ)], ps[:], float(2.0 ** KU) / W1SC, eng)

        def v_proj(lb, eng="alt"):
            # vt[l%P, lb, :] = (h W2)*2^KV rows for key block lb
            ps = psS.tile([P, C], f32, tag="s", name="ps_v")
            for cp in range(2):
                nc.tensor.matmul(ps[:], lhsT=hb[:, 2 * cp:2 * cp + 2, ts(lb, P)],
                                 rhs=wsb["w2"][:, 2 * cp:2 * cp + 2, :],
                                 start=(cp == 0), stop=(cp == 1), perf_mode=DR)
            epi(vt[:, lb, :], ps[:], float(2.0 ** KV) / W2SC, eng)

        if with_kappa:
            # kappa[m] = h_m . c3 + const  (per-key exp bias; only if bq != 0)
            psk = psS.tile([P, NLB], f32, tag="s", name="ps_k")
            for kb in range(NLB):
                for cp in range(2):
                    nc.tensor.matmul(psk[:, kb:kb + 1],
                                     lhsT=hb[:, 2 * cp:2 * cp + 2, ts(kb, P)],
                                     rhs=c3v[:, 2 * cp:2 * cp + 2, :],
                                     start=(cp == 0), stop=(cp == 1), perf_mode=DR)
            nc.vector.tensor_scalar(out=ksb[:], in0=psk[:], scalar1=kscv[:, 0:1],
                                    scalar2=kscv[:, 1:2], op0=ALU.mult, op1=ALU.add)

        u_proj(0)
        for lb in range(4):
            v_proj(lb)

        # ---- attention, per lq tile ----
        EXPS = float(2.0 ** -KU)

        def finale(lt, po, pd):
            # pd = 2^KV * denom ; rb = bcast(1/pd) ; out = po*rb + hbf
            # pdc on ACT: the DVE queue lags at finale time, ACT has slack
            pdc = small.tile([1, LQ], bf16, tag="pdc", bufs=2)
            with nc.allow_low_precision(reason="denom rounded to bf16 as matmul operand"):
                nc.scalar.activation(out=pdc[:], in_=pd[:], func=AF.Identity)
            pb = psS.tile([P, LQ], f32, tag="s", name="ps_b")
            nc.tensor.matmul(pb[:], lhsT=ones_row[:], rhs=pdc[:], start=True, stop=True)
            rb = finp.tile([P, LQ], f32, tag="rb")
            nc.vector.reciprocal_approx_fast(out=rb[:], in_=pb[:])
            for c_ in range(CS):
                t1 = t1p.tile([P, LQ], fp16, tag="t1")
                with nc.allow_low_precision(reason="attn term to fp16"):
                    nc.vector.tensor_tensor(out=t1[:], in0=po[c_][:], in1=rb[:], op=ALU.mult)
                ot = outp.tile([P, LQ], fp16, tag="ot")
                nc.vector.tensor_tensor(out=ot[:], in0=t1[:], in1=hbf[:, c_, ts(lt, LQ)],
                                        op=ALU.add)
                # sync queue only: a dependent dma_start on the ACT queue
                # would stall the exp instruction stream at the wait
                nc.sync.dma_start(out=out_dv[:, c_, ts(lt, LQ)], in_=ot[:])

        pending = None  # (lt, po, pd) awaiting finale emission
        for lt in range(NLT):
            po = [psA.tile([P, LQ], f32, tag="po", name=f"po{i}") for i in range(CS)]
            pd = psD.tile([1, LQ], f32, tag="d", name="pd")

            def pv_group(kp, pt2):
                for c_ in range(CS):
                    nc.tensor.matmul(po[c_][:], lhsT=vt[:, 2 * kp:2 * kp + 2, ts(c_, P)],
                                     rhs=pt2[:], start=(kp == 0), stop=(kp == NLB // 2 - 1),
                                     perf_mode=DR)
                nc.tensor.matmul(pd[:], lhsT=ones_col[:, :, 0:1], rhs=pt2[:],
                                 start=(kp == 0), stop=(kp == NLB // 2 - 1), perf_mode=DR)

            prev = None
            for kp in range(NLB // 2):
                pt2 = ptp.tile([P, 2, LQ], fp8, tag="pt")
                for i in range(2):
                    kb = 2 * kp + i
                    ps = psS.tile([P, LQ], f32, tag="s", name="ps_s")
                    for cp in range(2):
                        nc.tensor.matmul(ps[:], lhsT=hb[:, 2 * cp:2 * cp + 2, ts(kb, P)],
                                         rhs=ut[:, 2 * cp:2 * cp + 2, ts(lt, LQ)],
                                         start=(cp == 0), stop=(cp == 1), perf_mode=DR)
                    if with_kappa:
                        nc.scalar.activation(out=pt2[:, i, :], in_=ps[:], func=AF.Exp,
                                             scale=EXPS, bias=ksb[:, kb:kb + 1])
                    else:
                        nc.scalar.activation(out=pt2[:, i, :], in_=ps[:], func=AF.Exp,
                                             scale=EXPS)
                if kp == 0 and pending is not None:
                    # previous tile's finale: emitted after this tile's first
                    # S pair so the PE has queued work during the DVE chain
                    finale(*pending)
                    pending = None
                if lt == 0 and kp in (1, 2, 3):
                    # trailing hb slices (lt1 at kp1, lt2 at kp2, lt3 at kp3),
                    # needed by S/V'/u_proj from kp2/kp4/kp6 onward -- must be
                    # emitted BEFORE u_proj(lt+1) below, which reads them
                    for s in range(CS):
                        hb_slice(s, kp)
                if kp == 1 and lt < NLT - 1:
                    # next tile's U slice streamed into this tile's PE slack
                    u_proj(lt + 1, eng="dve")
                if prev is not None:
                    pv_group(*prev)
                prev = (kp, pt2)
                if lt == 0 and kp >= 2:
                    # V' blocks 4..15 interleaved two per kp; PV(kp) only
                    # needs blocks 2kp..2kp+1, produced one kp ahead
                    v_proj(2 * kp, eng="dve")
                    v_proj(2 * kp + 1, eng="dve")
            pv_group(*prev)
            pending = (lt, po, pd)
        finale(*pending)

    nc.compile()
    return nc


def get_nc(with_kappa):
    key = ("nc", with_kappa, W1SC, W2SC)
    if key not in _CACHE:
        _CACHE[key] = _build_nc(with_kappa)
    return _CACHE[key]


def _g0_const():
    g = np.zeros((P, 2), np.float32)
    g[0:CPG, 0] = 1.0 / CPG
    g[CPG:P, 1] = 1.0 / CPG
    return g


def _sel_const():
    s = np.zeros((2, P), np.float32)
    s[0, 0:CPG] = 1.0
    s[1, CPG:P] = 1.0
    return s


def _pow2_scale(w, target=240.0):
    """Largest power-of-2 s with max|w|*s <= target (fp8e4m3 headroom)."""
    m = float(np.abs(w).max())
    if m == 0.0:
        return 1.0
    return float(2.0 ** np.floor(np.log2(target / m)))


# module-level so _build_nc sees the host-chosen weight scales
W1SC = 1.0
W2SC = 1.0


def prep_inputs(x, gamma, beta, wq, bq, wk, bk, wv, bv, wp, bp):
    """Host-side prep: fold wq@wk^T and wv@wp, transpose/cast, per-core maps."""
    global W1SC, W2SC
    import ml_dtypes

    f = np.float32
    f8 = ml_dtypes.float8_e4m3fn
    x = np.asarray(x, f)
    lam = f(C) ** f(-0.5)

    w1 = (np.asarray(wq, f) @ np.asarray(wk, f).T) * lam      # S = h w1 h^T
    w2 = np.asarray(wv, f) @ np.asarray(wp, f)                # o' = h w2
    bres = np.asarray(bp, f) + np.asarray(bv, f) @ np.asarray(wp, f)
    W1SC = _pow2_scale(w1)
    W2SC = _pow2_scale(w2)
    with_kappa = bool(np.any(np.asarray(bq, f)))

    def wprep(w, s):
        w = np.asarray(w, f) * s
        return np.ascontiguousarray(w.reshape(CS, P, C).transpose(1, 0, 2)).astype(f8)

    def vprep(v):
        v = np.asarray(v, f)
        return np.ascontiguousarray(v.reshape(CS, P).T)

    shared = {
        "w1": wprep(w1, W1SC), "w2": wprep(w2, W2SC),
        "vp": np.ascontiguousarray(np.concatenate(
            [vprep(gamma), vprep(beta), vprep(bres)], axis=1)),
        "g0": _g0_const(), "sel": _sel_const(),
        "mg": np.full((2, CS), 0x5F3759DF, np.int32),
    }
    if with_kappa:
        # kappa[m] = lam * (h_m wk) . bq + lam * bq.bk  (per-key exp bias)
        c3 = lam * (np.asarray(wk, f) @ np.asarray(bq, f))    # [C]
        c3s = _pow2_scale(c3)
        shared["c3"] = np.ascontiguousarray(
            (c3 * c3s).reshape(CS, P).T.reshape(P, CS, 1)).astype(f8)
        kconst = lam * float(np.asarray(bq, f) @ np.asarray(bk, f))
        ksc = np.empty((P, 2), f)
        ksc[:, 0] = 1.0 / c3s
        ksc[:, 1] = kconst
        shared["ksc"] = ksc
    in_maps = []
    for b in range(N_CORES):
        m = dict(shared)
        # chunk-major [s*4+j, p, 512] so each 128KB chunk DMA is contiguous
        xt = x[b].T.astype(np.float16)                             # [C, L]
        m["xh"] = np.ascontiguousarray(
            xt.reshape(CS, P, 4, 512).transpose(0, 2, 1, 3).reshape(4 * CS, P, 512))
        in_maps.append(m)
    return in_maps, with_kappa


def run(inputs, trace=False, **kw):
    from concourse.bass_utils import run_bass_kernel_spmd

    in_maps, with_kappa = prep_inputs(**inputs)
    nc = get_nc(with_kappa)
    return run_bass_kernel_spmd(nc, in_maps, core_ids=list(range(N_CORES)),
                                trace=trace, **kw)


def kernel(**inputs) -> np.ndarray:
    res = run(inputs)
    out = np.empty((B, L, C), np.float32)
    for b in range(N_CORES):
        out[b] = np.asarray(res.results[b]["out_t"], np.float32).T
    return out


# revision 33
# speedup vs baseline: 1.2377x; 1.0185x over previous
"""Trainium2 Bass kernel for nn_AttentionBlock (B=8, L=2048, C=512, GroupNorm(8) +
single-head attention + residual), data-parallel over batch across 8 NeuronCores.

Self-contained: hardcodes shapes/sharding. kernel(**inputs) -> np.ndarray [B,L,C].

Two-matmul attention: the four projection weights collapse into two on the host
  W1 = wq @ wk^T / sqrt(C)     (S = h W1 h^T  -- q/k projections fused)
  W2 = wv @ wp                 (attn @ (h W2) -- v/output projections fused)
so the device computes, per core / batch element (channel-major h^T [C, L]):
  x^T fp16 --bn_stats/group-reduce--> a_c, b_c  (rstd via Newton-rsqrt on DVE,
    so the single ACT table set loaded at t=0 serves every activation)
  hb  = a*x + b                  (fp8, matmul operand; also the S lhsT = "K")
  hbf = a*x + b + (bp + bv@wp)   (fp16, residual + folded biases)
  U^T = W1^T h^T  (fp8)  ;  V' = h W2  (fp8, natural [L, C] layout)
  per 512-wide lq tile:
     for each 128-key block kb: S^T = hb-chunk^T @ U^T (PSUM); P = exp(S^T) fp8
     po += V'-chunk^T @ P  (PSUM accum) ; pd += (2^KV 1)^T @ P
     out^T = po * bcast(1/pd) + hbf     (fp16, DMA'd out)
Per-query bias terms cancel in softmax (exact); per-key terms (only if bq != 0)
ride the exp's per-partition bias.
"""

import numpy as np

B, L, C = 8, 2048, 512
GROUPS = 8
EPS = 1e-3
P = 128
CS = C // P            # 4 channel subtiles of 128
LQ = 512               # lq tile width (matmul free dim)
NLT = L // LQ          # 4 lq tiles
NLB = L // P           # 16 key/l blocks
CPG = C // GROUPS      # 64 channels per group
N_CORES = 8

_CACHE = {}


def _build_nc(with_kappa):
    from contextlib import ExitStack

    import concourse.bass as bass
    import concourse.mybir as mybir
    import concourse.tile as tile
    from concourse import bacc
    from concourse.bass import ts

    f32 = mybir.dt.float32
    f32r = mybir.dt.float32r
    fp16 = mybir.dt.float16
    bf16 = mybir.dt.bfloat16
    i32 = mybir.dt.int32
    fp8 = mybir.dt.float8e4
    DR = mybir.MatmulPerfMode.DoubleRow
    AF = mybir.ActivationFunctionType
    ALU = mybir.AluOpType

    nc = bacc.Bacc(trn_type="TRN2")

    # chunk-major [s*4+j, p, 512]: each 128KB DMA reads contiguous DRAM
    xh_d = nc.dram_tensor("xh", [4 * CS, P, 512], fp16, kind="ExternalInput")
    w_d = {
        n: nc.dram_tensor(n, [P, CS, C], fp8, kind="ExternalInput")
        for n in ("w1", "w2")
    }
    # packed per-channel vectors: [gamma, beta, bres] x CS columns
    vp_d = nc.dram_tensor("vp", [P, 3 * CS], f32, kind="ExternalInput")
    g0_d = nc.dram_tensor("g0", [P, 2], f32, kind="ExternalInput")
    sel_d = nc.dram_tensor("sel", [2, P], f32, kind="ExternalInput")
    mg_d = nc.dram_tensor("mg", [2, CS], i32, kind="ExternalInput")
    if with_kappa:
        c3_d = nc.dram_tensor("c3", [P, CS, 1], fp8, kind="ExternalInput")
        ksc_d = nc.dram_tensor("ksc", [P, 2], f32, kind="ExternalInput")
    out_d = nc.dram_tensor("out_t", [C, L], fp16, kind="ExternalOutput")

    out_dv = out_d[:].rearrange("(s p) l -> p s l", p=P)

    # scales (powers of two; host mirrors these exactly)
    KU = 5                 # ut = U * 2^KU
    KV = 2                 # vt = V' * 2^KV ; denom-ones = 2^KV so po/pd cancels

    with tile.TileContext(nc) as tc, ExitStack() as ctx:
        consts = ctx.enter_context(tc.tile_pool(name="consts", bufs=1))
        data = ctx.enter_context(tc.tile_pool(name="data", bufs=1))
        small = ctx.enter_context(tc.tile_pool(name="small", bufs=1))
        ptp = ctx.enter_context(tc.tile_pool(name="ptp", bufs=4))
        t1p = ctx.enter_context(tc.tile_pool(name="t1p", bufs=2))
        outp = ctx.enter_context(tc.tile_pool(name="outp", bufs=4))
        finp = ctx.enter_context(tc.tile_pool(name="finp", bufs=2))
        psA = ctx.enter_context(tc.tile_pool(name="psA", bufs=4, space="PSUM"))
        psS = ctx.enter_context(tc.tile_pool(name="psS", bufs=3, space="PSUM"))
        psD = ctx.enter_context(tc.tile_pool(name="psD", bufs=1, space="PSUM"))

        # ---- SBUF residents ----
        xh = data.tile([P, CS, L], fp16)      # x^T fp16
        hb = data.tile([P, CS, L], fp8)       # h^T fp8 (matmul operand + S lhsT)
        hbf = data.tile([P, CS, L], fp16)     # h^T + bres (residual, fp16)
        ut = data.tile([P, CS, L], fp8)       # U^T * 2^KU
        vt = data.tile([P, NLB, C], fp8)      # V' natural, [l%P, l//P, c] * 2^KV
        wsb = {n: consts.tile([P, CS, C], fp8, name=f"w_{n}") for n in w_d}
        vp = consts.tile([P, 3 * CS], f32)
        GAM, BET, BRES = (vp[:, i * CS:(i + 1) * CS] for i in range(3))
        g0 = consts.tile([P, 2], f32)
        sel = consts.tile([2, P], f32)
        ones_col = consts.tile([P, 2, 16], fp8)   # [:, :, 0:1] = 2^KV (DR pair)
        ones_row = consts.tile([1, P], bf16)
        eps2 = consts.tile([2, 1], f32)
        magic = consts.tile([2, CS], i32)
        if with_kappa:
            c3v = consts.tile([P, CS, 1], fp8)
            kscv = consts.tile([P, 2], f32)
            ksb = small.tile([P, NLB], f32)

        # ---- loads + constants ----
        # 512-wide chunks round-robined over the three DMA-capable queues so
        # bn_stats starts on the first chunk ~1us after the preamble.
        nc.gpsimd.dma_start(out=g0[:], in_=g0_d[:])
        nc.gpsimd.dma_start(out=sel[:], in_=sel_d[:])
        nc.gpsimd.dma_start(out=vp[:], in_=vp_d[:])
        nc.gpsimd.dma_start(out=magic[:], in_=mg_d[:])
        if with_kappa:
            nc.gpsimd.dma_start(out=c3v[:], in_=c3_d[:])
            nc.gpsimd.dma_start(out=kscv[:], in_=ksc_d[:])

        def xchunk(q, s, j):
            q.dma_start(out=xh[:, s, ts(j, 512)], in_=xh_d[4 * s + j])

        # per-queue order: first chunk of each bn_stats subtile leads (it
        # gates the DVE pipeline), s3 chunks next (ACT accum path)
        for j in range(4):
            xchunk(nc.sync, 0, j)
            xchunk(nc.scalar, 1, j)
            xchunk(nc.gpsimd, 2, j)
            if j == 0:
                xchunk(nc.sync, 3, 0)
                xchunk(nc.scalar, 3, 1)
                xchunk(nc.gpsimd, 3, 2)
            if j == 1:
                xchunk(nc.sync, 3, 3)
        nc.scalar.dma_start(out=wsb["w1"][:], in_=w_d["w1"][:])
        nc.scalar.dma_start(out=wsb["w2"][:], in_=w_d["w2"][:])
        nc.vector.memset(ones_col[:], float(2.0 ** KV))
        nc.vector.memset(ones_row[:], 1.0)
        nc.vector.memset(eps2[:], EPS)
        # warm the single ACT table set (exp_and_others: exp/identity/square)
        dm = small.tile([2, 1], f32, name="dm")
        nc.scalar.activation(out=dm[:], in_=eps2[:], func=AF.Exp)

        # ---- GroupNorm stats ----
        # per-channel (partition) sum / sumsq over L: subtiles 0-2 via DVE
        # bn_stats (chunk order matches DMA arrival); subtile 3 via ACT
        # activation accum (Identity / Square) on 1024-wide halves.
        st = small.tile([P, CS, 2], f32)      # (mean_c, E[x^2]_c) per subtile
        st6 = small.tile([P, 3, 4, 6], f32)
        for j in range(4):
            for s in range(3):
                nc.vector.bn_stats(out=st6[:, s, j, :], in_=xh[:, s, ts(j, 512)])
        gscr = small.tile([P, L], fp16)
        acc = small.tile([P, 2], f32)         # [p, (sum, sumsq)]
        nc.scalar.activation(out=gscr[:], in_=xh[:, 3, :], func=AF.Identity,
                             accum_out=acc[:, 0:1])
        nc.scalar.activation(out=gscr[:], in_=xh[:, 3, :], func=AF.Square,
                             accum_out=acc[:, 1:2])
        for s in range(3):
            mv = small.tile([P, 2], f32, tag="mv", bufs=2)
            nc.vector.bn_aggr(out=mv[:], in_=st6[:, s, :, :])
            nc.vector.tensor_copy(out=st[:, s, 0:1], in_=mv[:, 0:1])
            nc.vector.tensor_tensor(out=st[:, s, 1:2], in0=mv[:, 0:1], in1=mv[:, 0:1], op=ALU.mult)
            nc.vector.tensor_tensor(out=st[:, s, 1:2], in0=st[:, s, 1:2], in1=mv[:, 1:2], op=ALU.add)
        nc.vector.tensor_scalar(out=st[:, 3, :], in0=acc[:], scalar1=1.0 / L, scalar2=None,
                                op0=ALU.mult)

        psg = psD.tile([2, 2 * CS], f32, tag="d")   # [group-half, (s, stat)]
        nc.tensor.matmul(psg[:], lhsT=g0[:], rhs=st[:].rearrange("p a b -> p (a b)"),
                         start=True, stop=True)
        pst = small.tile([2, 2 * CS], f32)
        nc.vector.tensor_copy(out=pst[:], in_=psg[:])
        pstv = pst[:].rearrange("p (s k) -> p s k", k=2)
        msq = small.tile([2, CS], f32)
        nc.vector.tensor_tensor(out=msq[:], in0=pstv[:, :, 0], in1=pstv[:, :, 0], op=ALU.mult)
        grp = small.tile([2, 2 * CS], f32)     # [:, :CS]=rstd_g, [:, CS:]=mean_g
        vv = small.tile([2, CS], f32)          # var + eps
        nc.vector.tensor_tensor(out=vv[:], in0=pstv[:, :, 1], in1=msq[:], op=ALU.subtract)
        nc.vector.tensor_scalar(out=vv[:], in0=vv[:], scalar1=1.0, scalar2=EPS,
                                op0=ALU.mult, op1=ALU.add)
        # rstd = rsqrt(var+eps): Quake bit-trick seed + 2 Newton iterations,
        # entirely on DVE -- no Sqrt/Ln table switches on the ACT engine.
        nc.vector.tensor_copy(out=grp[:, CS:], in_=pstv[:, :, 0])
        y = grp[:, 0:CS]
        yi = y.bitcast(i32)
        nc.vector.tensor_scalar(out=yi, in0=vv[:].bitcast(i32), scalar1=1, scalar2=None,
                                op0=ALU.logical_shift_right)
        nc.vector.tensor_tensor(out=yi, in0=magic[:], in1=yi, op=ALU.subtract)
        tn = small.tile([2, CS], f32)
        for _ in range(2):
            nc.vector.tensor_tensor(out=tn[:], in0=vv[:], in1=y, op=ALU.mult)
            nc.vector.tensor_tensor(out=tn[:], in0=tn[:], in1=y, op=ALU.mult)
            nc.vector.tensor_scalar(out=tn[:], in0=tn[:], scalar1=-0.5, scalar2=1.5,
                                    op0=ALU.mult, op1=ALU.add)
            nc.vector.tensor_tensor(out=y, in0=y, in1=tn[:], op=ALU.mult)

        psbc = psD.tile([P, 2 * CS], f32, tag="d")  # broadcast groups -> channels
        nc.tensor.matmul(psbc[:], lhsT=sel[:], rhs=grp[:], start=True, stop=True)
        ab = small.tile([P, 2 * CS], f32)      # [:, :CS]=a_c, [:, CS:]=b_c
        nc.vector.tensor_tensor(out=ab[:, 0:CS], in0=GAM, in1=psbc[:, 0:CS], op=ALU.mult)
        nc.vector.tensor_tensor(out=ab[:, CS:], in0=psbc[:, CS:], in1=ab[:, 0:CS], op=ALU.mult)
        nc.vector.tensor_tensor(out=ab[:, CS:], in0=BET, in1=ab[:, CS:], op=ALU.subtract)
        # residual-pass intercept: b + bres (bres = bp + bv @ wp, host-folded)
        ab2 = small.tile([P, CS], f32)
        nc.vector.tensor_tensor(out=ab2[:], in0=ab[:, CS:], in1=BRES, op=ALU.add)

        # ---- normalize ----
        # hb (fp8 matmul operand) per (subtile, lt-slice) so the first U/V'
        # matmuls start after 4 small ops; split DVE/ACT. hbf (fp16 residual)
        # only feeds finales: lt0 early on DVE, lt1-3 on GpSimd.
        def hb_slice(s, lt):
            if (s + lt) % 2 == 0:
                nc.vector.tensor_scalar(out=hb[:, s, ts(lt, LQ)], in0=xh[:, s, ts(lt, LQ)],
                                        scalar1=ab[:, s:s + 1], scalar2=ab[:, CS + s:CS + s + 1],
                                        op0=ALU.mult, op1=ALU.add)
            else:
                nc.scalar.activation(out=hb[:, s, ts(lt, LQ)], in_=xh[:, s, ts(lt, LQ)],
                                     func=AF.Identity,
                                     bias=ab[:, CS + s:CS + s + 1], scale=ab[:, s:s + 1])

        # only lt0's hb slices gate the first U/V'/S matmuls; lt1/lt2 slices
        # are emitted inside lt0's kp loop (DVE+ACT slack), lt3 + all of hbf
        # on GpSimd (first needed at kp6 / the first finale respectively)
        for s in range(CS):
            hb_slice(s, 0)
        for lt in range(NLT):
            for s in range(CS):
                nc.gpsimd.tensor_scalar(out=hbf[:, s, ts(lt, LQ)], in0=xh[:, s, ts(lt, LQ)],
                                        scalar1=ab[:, s:s + 1], scalar2=ab2[:, s:s + 1],
                                        op0=ALU.mult, op1=ALU.add)

        # ---- projections ----
        epi_ix = [0]

        def epi(dst, src, scl, eng):
            # PSUM -> SBUF fp8 cast with scale. 'alt' alternates DVE / ACT
            # (pre-attention, both engines have slack); 'dve' keeps the ACT
            # queue free for the exp stream during attention.
            if eng == "alt":
                eng = "dve" if epi_ix[0] % 2 == 0 else "act"
                epi_ix[0] += 1
            if eng == "dve":
                nc.vector.tensor_scalar(out=dst, in0=src, scalar1=scl, scalar2=None,
                                        op0=ALU.mult)
            else:
                nc.scalar.activation(out=dst, in_=src, func=AF.Identity, scale=scl)

        def u_proj(lt, eng="alt"):
            # ut[:, co_s, lq] = sum_ci W1[ci, co]^T h^T ; scale 2^KU / W1SC
            for co_s in range(CS):
                ps = psS.tile([P, LQ], f32, tag="s", name="ps_u")
                for cp in range(2):
                    nc.tensor.matmul(ps[:], lhsT=wsb["w1"][:, 2 * cp:2 * cp + 2, ts(co_s, P)],
                                     rhs=hb[:, 2 * cp:2 * cp + 2, ts(lt, LQ)],
                                     start=(cp == 0), stop=(cp == 1), perf_mode=DR)
                epi(ut[:, co_s, ts(lt, LQ)], ps[:], float(2.0 ** KU) / W1SC, eng)

        def v_proj(lb, eng="alt"):
            # vt[l%P, lb, :] = (h W2)*2^KV rows for key block lb
            ps = psS.tile([P, C], f32, tag="s", name="ps_v")
            for cp in range(2):
                nc.tensor.matmul(ps[:], lhsT=hb[:, 2 * cp:2 * cp + 2, ts(lb, P)],
                                 rhs=wsb["w2"][:, 2 * cp:2 * cp + 2, :],
                                 start=(cp == 0), stop=(cp == 1), perf_mode=DR)
            epi(vt[:, lb, :], ps[:], float(2.0 ** KV) / W2SC, eng)

        if with_kappa:
            # kappa[m] = h_m . c3 + const  (per-key exp bias; only if bq != 0)
            psk = psS.tile([P, NLB], f32, tag="s", name="ps_k")
            for kb in range(NLB):
                for cp in range(2):
                    nc.tensor.matmul(psk[:, kb:kb + 1],
                                     lhsT=hb[:, 2 * cp:2 * cp + 2, ts(kb, P)],
                                     rhs=c3v[:, 2 * cp:2 * cp + 2, :],
                                     start=(cp == 0), stop=(cp == 1), perf_mode=DR)
            nc.vector.tensor_scalar(out=ksb[:], in0=psk[:], scalar1=kscv[:, 0:1],
                                    scalar2=kscv[:, 1:2], op0=ALU.mult, op1=ALU.add)

        u_proj(0)
        for lb in range(4):
            v_proj(lb)

        # ---- attention, per lq tile ----
        EXPS = float(2.0 ** -KU)

        def finale(lt, po, pd):
            # pd = 2^KV * denom ; rb = bcast(1/pd) ; out = po*rb + hbf
            # pdc on ACT: the DVE queue lags at finale time, ACT has slack
            pdc = small.tile([1, LQ], bf16, tag="pdc", bufs=2)
            with nc.allow_low_precision(reason="denom rounded to bf16 as matmul operand"):
                nc.scalar.activation(out=pdc[:], in_=pd[:], func=AF.Identity)
            pb = psS.tile([P, LQ], f32, tag="s", name="ps_b")
            nc.tensor.matmul(pb[:], lhsT=ones_row[:], rhs=pdc[:], start=True, stop=True)
            rb = finp.tile([P, LQ], f32, tag="rb")
            nc.vector.reciprocal_approx_fast(out=rb[:], in_=pb[:])
            for c_ in range(CS):
                t1 = t1p.tile([P, LQ], fp16, tag="t1")
                with nc.allow_low_precision(reason="attn term to fp16"):
                    nc.vector.tensor_tensor(out=t1[:], in0=po[c_][:], in1=rb[:], op=ALU.mult)
                ot = outp.tile([P, LQ], fp16, tag="ot")
                nc.vector.tensor_tensor(out=ot[:], in0=t1[:], in1=hbf[:, c_, ts(lt, LQ)],
                                        op=ALU.add)
                # sync/gpsimd queues only (both idle here): a dependent
                # dma_start on the ACT queue would stall the exp stream
                q = nc.sync if c_ % 2 == 0 else nc.gpsimd
                q.dma_start(out=out_dv[:, c_, ts(lt, LQ)], in_=ot[:])

        pending = None  # (lt, po, pd) awaiting finale emission
        for lt in range(NLT):
            po = [psA.tile([P, LQ], f32, tag="po", name=f"po{i}") for i in range(CS)]
            pd = psD.tile([1, LQ], f32, tag="d", name="pd")

            def pv_group(kp, pt2):
                for c_ in range(CS):
                    nc.tensor.matmul(po[c_][:], lhsT=vt[:, 2 * kp:2 * kp + 2, ts(c_, P)],
                                     rhs=pt2[:], start=(kp == 0), stop=(kp == NLB // 2 - 1),
                                     perf_mode=DR)
                nc.tensor.matmul(pd[:], lhsT=ones_col[:, :, 0:1], rhs=pt2[:],
                                 start=(kp == 0), stop=(kp == NLB // 2 - 1), perf_mode=DR)

            prev = None
            for kp in range(NLB // 2):
                pt2 = ptp.tile([P, 2, LQ], fp8, tag="pt")
                for i in range(2):
                    kb = 2 * kp + i
                    ps = psS.tile([P, LQ], f32, tag="s", name="ps_s")
                    for cp in range(2):
                        nc.tensor.matmul(ps[:], lhsT=hb[:, 2 * cp:2 * cp + 2, ts(kb, P)],
                                         rhs=ut[:, 2 * cp:2 * cp + 2, ts(lt, LQ)],
                                         start=(cp == 0), stop=(cp == 1), perf_mode=DR)
                    if with_kappa:
                        nc.scalar.activation(out=pt2[:, i, :], in_=ps[:], func=AF.Exp,
                                             scale=EXPS, bias=ksb[:, kb:kb + 1])
                    else:
                        nc.scalar.activation(out=pt2[:, i, :], in_=ps[:], func=AF.Exp,
                                             scale=EXPS)
                if kp == 0 and pending is not None:
                    # previous tile's finale: emitted after this tile's first
                    # S pair so the PE has queued work during the DVE chain
                    finale(*pending)
                    pending = None
                if lt == 0 and kp in (1, 2, 3):
                    # trailing hb slices (lt1 at kp1, lt2 at kp2, lt3 at kp3),
                    # needed by S/V'/u_proj from kp2/kp4/kp6 onward -- must be
                    # emitted BEFORE u_proj(lt+1) below, which reads them
                    for s in range(CS):
                        hb_slice(s, kp)
                if kp == 1 and lt < NLT - 1:
                    # next tile's U slice streamed into this tile's PE slack
                    u_proj(lt + 1, eng="dve")
                if prev is not None:
                    pv_group(*prev)
                prev = (kp, pt2)
                if lt == 0 and kp >= 2:
                    # V' blocks 4..15 interleaved two per kp; PV(kp) only
                    # needs blocks 2kp..2kp+1, produced one kp ahead
                    v_proj(2 * kp, eng="dve")
                    v_proj(2 * kp + 1, eng="dve")
            pv_group(*prev)
            pending = (lt, po, pd)
        finale(*pending)

    nc.compile()
    return nc


def get_nc(with_kappa):
    key = ("nc", with_kappa, W1SC, W2SC)
    if key not in _CACHE:
        _CACHE[key] = _build_nc(with_kappa)
    return _CACHE[key]


def _g0_const():
    g = np.zeros((P, 2), np.float32)
    g[0:CPG, 0] = 1.0 / CPG
    g[CPG:P, 1] = 1.0 / CPG
    return g


def _sel_const():
    s = np.zeros((2, P), np.float32)
    s[0, 0:CPG] = 1.0
    s[1, CPG:P] = 1.0
    return s


def _pow2_scale(w, target=240.0):
    """Largest power-of-2 s with max|w|*s <= target (fp8e4m3 headroom)."""
    m = float(np.abs(w).max())
    if m == 0.0:
        return 1.0
    return float(2.0 ** np.floor(np.log2(target / m)))


# module-level so _build_nc sees the host-chosen weight scales
W1SC = 1.0
W2SC = 1.0


def prep_inputs(x, gamma, beta, wq, bq, wk, bk, wv, bv, wp, bp):
    """Host-side prep: fold wq@wk^T and wv@wp, transpose/cast, per-core maps."""
    global W1SC, W2SC
    import ml_dtypes

    f = np.float32
    f8 = ml_dtypes.float8_e4m3fn
    x = np.asarray(x, f)
    lam = f(C) ** f(-0.5)

    w1 = (np.asarray(wq, f) @ np.asarray(wk, f).T) * lam      # S = h w1 h^T
    w2 = np.asarray(wv, f) @ np.asarray(wp, f)                # o' = h w2
    bres = np.asarray(bp, f) + np.asarray(bv, f) @ np.asarray(wp, f)
    W1SC = _pow2_scale(w1)
    W2SC = _pow2_scale(w2)
    with_kappa = bool(np.any(np.asarray(bq, f)))

    def wprep(w, s):
        w = np.asarray(w, f) * s
        return np.ascontiguousarray(w.reshape(CS, P, C).transpose(1, 0, 2)).astype(f8)

    def vprep(v):
        v = np.asarray(v, f)
        return np.ascontiguousarray(v.reshape(CS, P).T)

    shared = {
        "w1": wprep(w1, W1SC), "w2": wprep(w2, W2SC),
        "vp": np.ascontiguousarray(np.concatenate(
            [vprep(gamma), vprep(beta), vprep(bres)], axis=1)),
        "g0": _g0_const(), "sel": _sel_const(),
        "mg": np.full((2, CS), 0x5F3759DF, np.int32),
    }
    if with_kappa:
        # kappa[m] = lam * (h_m wk) . bq + lam * bq.bk  (per-key exp bias)
        c3 = lam * (np.asarray(wk, f) @ np.asarray(bq, f))    # [C]
        c3s = _pow2_scale(c3)
        shared["c3"] = np.ascontiguousarray(
            (c3 * c3s).reshape(CS, P).T.reshape(P, CS, 1)).astype(f8)
        kconst = lam * float(np.asarray(bq, f) @ np.asarray(bk, f))
        ksc = np.empty((P, 2), f)
        ksc[:, 0] = 1.0 / c3s
        ksc[:, 1] = kconst
        shared["ksc"] = ksc
    in_maps = []
    for b in range(N_CORES):
        m = dict(shared)
        # chunk-major [s*4+j, p, 512] so each 128KB chunk DMA is contiguous
        xt = x[b].T.astype(np.float16)                             # [C, L]
        m["xh"] = np.ascontiguousarray(
            xt.reshape(CS, P, 4, 512).transpose(0, 2, 1, 3).reshape(4 * CS, P, 512))
        in_maps.append(m)
    return in_maps, with_kappa


def run(inputs, trace=False, **kw):
    from concourse.bass_utils import run_bass_kernel_spmd

    in_maps, with_kappa = prep_inputs(**inputs)
    nc = get_nc(with_kappa)
    return run_bass_kernel_spmd(nc, in_maps, core_ids=list(range(N_CORES)),
                                trace=trace, **kw)


def kernel(**inputs) -> np.ndarray:
    res = run(inputs)
    out = np.empty((B, L, C), np.float32)
    for b in range(N_CORES):
        out[b] = np.asarray(res.results[b]["out_t"], np.float32).T
    return out
